# revision 1
# baseline (speedup 1.0000x reference)
"""Trainium2 Bass kernel for multi-head causal self-attention.

Problem: B=4, T=1024, D=2048, H=16 heads, E=128 head_dim, fp32 I/O.
  q/k/v = per-head projections of x; scores = causal-masked softmax(q k^T / sqrt(E));
  y = probs @ v; out = concat-heads(y) @ Wo^T + bo.

Sharding: 8 cores = 4 batches x 2 head-halves. Core c handles batch c//2 and
heads (c%2)*8 .. (c%2)*8+7. Each core computes its heads' q/k/v projections,
attention, and a partial out-projection (y_span @ Wo[:, span]^T) -> [D, T]
partial transposed output. Host sums the two half partials per batch and adds
the folded bias.

Bias folding (host side):
  - bk: adds a per-query constant to every score row -> softmax-invariant -> dropped.
  - bv: rows of probs sum to 1, so v-bias passes through attention additively ->
        folded into bo_total = bo + Wo @ concat(bv).
  - bq: applied on-device during q eviction (scaled).

Scores are computed pre-transposed per key-tile (k stationary, q moving), so
exp writes the P@V moving operand (ET) straight from PSUM to SBUF -- no PE
transposes or PSUM evictions of probabilities. Row sums arrive already
broadcast across partitions via an extra all-ones matmul riding next to the
y=P@V accumulation; softmax normalization (1/r) is deferred to the y PSUM
eviction. Nothing in the softmax tail ever gates the TensorE stream.

All matmuls run as float32r (~tf32 precision, full PE rate at N>=256) with fp32
PSUM accumulation. Measured end-to-end accuracy vs fp32 reference ~2e-4.
"""

import numpy as np

B, T, D, H = 4, 1024, 2048, 16
E = D // H            # 128
P = 128
ND = D // P           # 16 d-tiles
NT = T // P           # 8 t-blocks / q-blocks / k-tiles
HL = H // 2           # 8 heads per core
SCALE = 1.0 / np.sqrt(E)

_cache = {}


def _build():
    import concourse.bass as bass
    import concourse.mybir as mybir
    import concourse.tile as tile
    from concourse import bacc
    from concourse.bass import ts
    from concourse.masks import make_identity, make_causal_mask

    F32 = mybir.dt.float32
    F32R = mybir.dt.float32r
    AF = mybir.ActivationFunctionType
    OP = mybir.AluOpType

    nc = bacc.Bacc("TRN2", target_bir_lowering=False, debug=False)

    xT_d = nc.dram_tensor("xT", [ND, P, T], F32R, kind="ExternalInput").ap()
    w3_d = nc.dram_tensor("w3", [HL, 4, P, 4 * 3 * P], F32R, kind="ExternalInput").ap()
    bqT_d = nc.dram_tensor("bqT", [P, HL], F32, kind="ExternalInput").ap()
    wot_d = nc.dram_tensor("wot", [ND, P, HL * P], F32R, kind="ExternalInput").ap()
    outT_d = nc.dram_tensor("outT", [ND, P, T], F32, kind="ExternalOutput").ap()

    with tile.TileContext(nc) as tc:
        with (
            tc.tile_pool(name="const", bufs=1) as const,
            tc.tile_pool(name="qkv", bufs=1) as qkv,
            tc.tile_pool(name="small", bufs=4) as small,
        ):
            ident = const.tile([P, P], F32)
            make_identity(nc, ident[:])
            ident_r = const.tile([P, P], F32R)
            nc.vector.tensor_copy(ident_r[:], ident[:])
            ones_f = const.tile([P, P], F32)
            nc.vector.memset(ones_f[:], 1.0)
            ones_r = const.tile([P, P], F32R)
            nc.vector.tensor_copy(ones_r[:], ones_f[:])
            bqT_t = const.tile([P, HL], F32)

            qT = qkv.tile([P, HL, T], F32R, tag="qT")   # [e, head, t]
            kT = qkv.tile([P, HL, T], F32R, tag="kT")
            vT = qkv.tile([P, HL, T], F32R, tag="vT")

            # ---------------- Phase A: q/k/v projections ----------------
            with (
                tc.tile_pool(name="xTp", bufs=1) as xTp,
                tc.tile_pool(name="w3p", bufs=4) as w3p,
                tc.tile_pool(name="ps_a", bufs=4, space="PSUM") as ps_a,
            ):
                xT_t = xTp.tile([P, ND, T], F32R)

                def w3_dma(hl, g, split=False):
                    w3t = w3p.tile([P, 4, 3 * P], F32R, tag="w3", name="w3t")
                    src_ap = w3_d[hl, g].rearrange("p (t f) -> p t f", t=4)
                    if split:  # per-d-tile pieces so the first matmul starts asap
                        for u in range(4):
                            nc.sync.dma_start(w3t[:, u, :], src_ap[:, u, :])
                    else:
                        nc.sync.dma_start(w3t[:], src_ap)
                    return w3t

                # interleave head-0 weights with xT so compute starts immediately
                w3_h0 = [w3_dma(0, 0)]
                nc.sync.dma_start(xT_t[:, 0, :], xT_d[0])
                for g in range(4):
                    if g > 0:
                        w3_h0.append(w3_dma(0, g))
                    for dt in range(max(4 * g, 1), 4 * g + 4):
                        nc.sync.dma_start(xT_t[:, dt, :], xT_d[dt])

                nc.sync.dma_start(bqT_t[:], bqT_d)
                # preload the ACT Exp table off the critical path
                dummy = small.tile([P, 1], F32, tag="racc", name="dummy")
                nc.scalar.activation(dummy[:], bqT_t[:, 0:1], AF.Exp)

                for hl in range(HL):
                    w3g = w3_h0 if hl == 0 else [w3_dma(hl, g) for g in range(4)]
                    psq = ps_a.tile([P, T], F32, tag="a")
                    psk = ps_a.tile([P, T], F32, tag="a")
                    psv = ps_a.tile([P, T], F32, tag="a")
                    for dt in range(ND):
                        w3t = w3g[dt // 4]
                        dl = dt % 4
                        st, sp = dt == 0, dt == ND - 1
                        for c in range(2):
                            rhs = xT_t[:, dt, ts(c, 512)]
                            nc.tensor.matmul(psq[:, ts(c, 512)], w3t[:, dl, 0:P], rhs,
                                             start=st, stop=sp)
                            nc.tensor.matmul(psk[:, ts(c, 512)], w3t[:, dl, P:2 * P], rhs,
                                             start=st, stop=sp)
                            nc.tensor.matmul(psv[:, ts(c, 512)], w3t[:, dl, 2 * P:3 * P], rhs,
                                             start=st, stop=sp)
                    # evict: q gets (psum + bq) * scale on DVE; k/v copies on ACT
                    nc.vector.tensor_scalar(
                        qT[:, hl, :], psq[:], bqT_t[:, hl : hl + 1], float(SCALE),
                        op0=OP.add, op1=OP.mult,
                    )
                    nc.scalar.activation(kT[:, hl, :], psk[:], AF.Copy)
                    nc.vector.tensor_copy(vT[:, hl, :], psv[:])

            # ---------------- Phases B+C scope ----------------
            with (
                tc.tile_pool(name="yTp", bufs=1) as yTp,
                tc.tile_pool(name="wop", bufs=3) as wop,
            ):
                yT = yTp.tile([P, HL, T], F32R)  # [e, head(=i-tile), t]

                def wot_dma(ob):
                    wot_t = wop.tile([P, HL, P], F32R, tag="wo", name="wot_t")
                    nc.sync.dma_start(
                        wot_t[:], wot_d[ob].rearrange("p (i f) -> p i f", i=HL)
                    )
                    return wot_t

                wot_pre = [wot_dma(ob) for ob in range(2)]

                # -------- Phase B: attention per head, scores computed
                # pre-transposed (k stationary, q moving) so exp writes the
                # AV moving operand ET straight to SBUF. Row sums r ride as an
                # extra all-ones matmul that lands already broadcast across
                # partitions; 1/r multiplies in at the y eviction. --------
                with (
                    tc.tile_pool(name="etp", bufs=1) as etp,
                    tc.tile_pool(name="vhp", bufs=2) as vhp,
                    tc.tile_pool(name="rbp", bufs=2) as rbp,
                    tc.tile_pool(name="ps_s", bufs=4, space="PSUM") as ps_s,
                    tc.tile_pool(name="ps_ar", bufs=2, space="PSUM") as ps_ar,
                ):
                    ev_cnt = [0]

                    def evict(out_ap, in_ap):
                        # alternate PSUM->SBUF eviction between DVE and ACT
                        ev_cnt[0] += 1
                        if ev_cnt[0] % 2:
                            nc.vector.tensor_copy(out_ap, in_ap)
                        else:
                            nc.scalar.activation(out_ap, in_ap, AF.Copy)

                    v_next = [None]
                    tail_work = [None]
                    for hl in range(HL):
                        if hl == 0:
                            v_h = vhp.tile([P, NT, P], F32R, tag="vh")
                        else:
                            v_h = v_next[0]
                        ET = etp.tile([P, NT, T], F32R, tag="ET")  # [k, k-tile, q]
                        rb = rbp.tile([P, T], F32, tag="rb")       # 1/r rows

                        def emit_vpair(jp, v_h=v_h, hl=hl):
                            t_ps = ps_s.tile([P, 2, P], F32R, tag="s", name="t_ps")
                            for u in range(2):
                                nc.tensor.matmul(
                                    t_ps[:, u, :], vT[:, hl, ts(2 * jp + u, P)],
                                    ident_r[:], is_transpose=True,
                                    skip_group_check=True,
                                )
                            evict(v_h[:, 2 * jp:2 * jp + 2, :], t_ps[:])

                        def emit_ST(j, ET=ET, hl=hl):
                            kblk = kT[:, hl, ts(j, P)]
                            if j < 4:
                                wA = 512 - j * P
                                sA = ps_s.tile([P, 512], F32, tag="s", name="sA")
                                nc.tensor.matmul(sA[:, 0:wA], kblk,
                                                 qT[:, hl, j * P:512],
                                                 start=True, stop=True)
                                sB = ps_s.tile([P, 512], F32, tag="s", name="sB")
                                nc.tensor.matmul(sB[:], kblk, qT[:, hl, 512:T],
                                                 start=True, stop=True)
                                nc.scalar.activation(ET[:, j, j * P:(j + 1) * P],
                                                     sA[:, 0:P], AF.Exp)
                                nc.gpsimd.affine_select(
                                    out=ET[:, j, j * P:(j + 1) * P],
                                    in_=ET[:, j, j * P:(j + 1) * P],
                                    compare_op=mybir.AluOpType.is_ge, fill=0.0,
                                    base=0, pattern=[[1, P]], channel_multiplier=-1,
                                )
                                if wA > P:
                                    nc.scalar.activation(ET[:, j, (j + 1) * P:512],
                                                         sA[:, P:wA], AF.Exp)
                                nc.scalar.activation(ET[:, j, 512:T], sB[:], AF.Exp)
                            else:
                                lo = j * P - 512
                                sB = ps_s.tile([P, 512], F32, tag="s", name="sB")
                                nc.tensor.matmul(sB[:, lo:512], kblk,
                                                 qT[:, hl, j * P:T],
                                                 start=True, stop=True)
                                nc.scalar.activation(ET[:, j, j * P:(j + 1) * P],
                                                     sB[:, lo:lo + P], AF.Exp)
                                nc.gpsimd.affine_select(
                                    out=ET[:, j, j * P:(j + 1) * P],
                                    in_=ET[:, j, j * P:(j + 1) * P],
                                    compare_op=mybir.AluOpType.is_ge, fill=0.0,
                                    base=0, pattern=[[1, P]], channel_multiplier=-1,
                                )
                                if j < NT - 1:
                                    nc.scalar.activation(ET[:, j, (j + 1) * P:T],
                                                         sB[:, lo + P:512], AF.Exp)

                        # y and r accumulate per 512-wide q-chunk:
                        #   ar0 <- q in [0,512) over j<=3; ar1 <- [512,1024) over all j
                        y0 = ps_ar.tile([P, 512], F32, tag="y", name="y0")
                        r0 = ps_ar.tile([P, 512], F32, tag="r", name="r0")
                        y1 = ps_ar.tile([P, 512], F32, tag="y", name="y1")
                        r1 = ps_ar.tile([P, 512], F32, tag="r", name="r1")

                        def emit_AVR(jq, y0=y0, r0=r0, y1=y1, r1=r1, v_h=v_h, ET=ET):
                            if jq <= 3:
                                lo = jq * P
                                st, sp = jq == 0, jq == 3
                                nc.tensor.matmul(y0[:, lo:512], v_h[:, jq, :],
                                                 ET[:, jq, lo:512], start=st, stop=sp,
                                                 skip_group_check=True)
                                nc.tensor.matmul(r0[:, lo:512], ones_r[:],
                                                 ET[:, jq, lo:512], start=st, stop=sp,
                                                 skip_group_check=True)
                            lo = max(jq * P, 512)
                            st, sp = jq == 0, jq == NT - 1
                            nc.tensor.matmul(y1[:, lo - 512:512], v_h[:, jq, :],
                                             ET[:, jq, lo:T], start=st, stop=sp,
                                             skip_group_check=True)
                            nc.tensor.matmul(r1[:, lo - 512:512], ones_r[:],
                                             ET[:, jq, lo:T], start=st, stop=sp,
                                             skip_group_check=True)

                        for j in range(NT):
                            if hl == 0 and j <= 3:
                                emit_vpair(j)
                            emit_ST(j)
                            if j == 1 and tail_work[0] is not None:
                                tail_work[0]()
                                tail_work[0] = None
                            if j >= 2:
                                emit_AVR(j - 2)
                            if j == 5:
                                nc.vector.reciprocal(rb[:, 0:512], r0[:])
                                nc.vector.tensor_mul(yT[:, hl, 0:512],
                                                     y0[:], rb[:, 0:512])
                        emit_AVR(NT - 2)
                        # prefetch next head's v transposes into this head's tail
                        if hl + 1 < HL:
                            nv = vhp.tile([P, NT, P], F32R, tag="vh", name="nv")
                            v_next[0] = nv
                            for jp in range(4):
                                emit_vpair(jp, v_h=nv, hl=hl + 1)
                        emit_AVR(NT - 1)

                        def tail(y1=y1, r1=r1, rb=rb, hl=hl):
                            nc.vector.reciprocal(rb[:, 512:T], r1[:])
                            nc.vector.tensor_mul(yT[:, hl, 512:T],
                                                 y1[:], rb[:, 512:T])

                        if hl + 1 < HL:
                            tail_work[0] = tail
                        else:
                            tail()

                # -------- Phase C: partial out-projection --------
                with (
                    tc.tile_pool(name="osb", bufs=2) as osb,
                    tc.tile_pool(name="ps_o", bufs=2, space="PSUM") as ps_o,
                ):
                    for ob in range(ND):
                        wot_t = wot_pre[ob] if ob < 2 else wot_dma(ob)
                        o_ps = ps_o.tile([P, T], F32, tag="o")
                        for it in range(HL):
                            st, sp = it == 0, it == HL - 1
                            for c in range(2):
                                nc.tensor.matmul(
                                    o_ps[:, ts(c, 512)], wot_t[:, it, :],
                                    yT[:, it, ts(c, 512)], start=st, stop=sp,
                                )
                        out_sb = osb.tile([P, T], F32, tag="osb")
                        nc.scalar.activation(out_sb[:], o_ps[:], AF.Copy)
                        nc.sync.dma_start(outT_d[ob], out_sb[:])

    nc.compile()
    return nc


def _get_compiled():
    if "nc" not in _cache:
        _cache["nc"] = _build()
    return _cache["nc"]


def _host_prep(x, Wq, bq, Wk, Wv, Wo):
    """Build per-core input maps."""
    in_maps = []
    # xT per batch: [D, T] -> [ND, P, T]
    xTs = [np.ascontiguousarray(x[b].T).reshape(ND, P, T) for b in range(B)]
    halves = []
    for half in range(2):
        hs = slice(half * HL, (half + 1) * HL)
        # WqT/WkT/WvT per head: [D, E] -> [ND, P, E]; pack so each 4-d-tile
        # group is one contiguous [P, 4*3P] DMA: [HL, 4, P, 4*3P]
        w3s = np.empty((HL, ND, P, 3 * P), dtype=np.float32)
        for hl, h in enumerate(range(half * HL, (half + 1) * HL)):
            w3s[hl, :, :, 0:P] = Wq[h].T.reshape(ND, P, P)
            w3s[hl, :, :, P:2 * P] = Wk[h].T.reshape(ND, P, P)
            w3s[hl, :, :, 2 * P:3 * P] = Wv[h].T.reshape(ND, P, P)
        w3 = np.ascontiguousarray(
            w3s.reshape(HL, 4, 4, P, 3 * P).transpose(0, 1, 3, 2, 4)
        ).reshape(HL, 4, P, 4 * 3 * P)
        bqT = np.ascontiguousarray(bq[hs].T)  # [E, HL]
        # WoT span blocks: WoT = Wo.T [i, o]; rows i in this half's span
        WoT_span = Wo.T[half * 1024:(half + 1) * 1024]  # [1024, D]
        # pack to [ND(o-block), P, HL*P] so each o-block is one contiguous DMA
        wot = np.ascontiguousarray(
            WoT_span.reshape(HL, P, ND, P).transpose(2, 1, 0, 3)
        ).reshape(ND, P, HL * P)
        halves.append({"w3": w3, "bqT": bqT, "wot": wot})
    for c in range(8):
        b, half = c // 2, c % 2
        hv = halves[half]
        in_maps.append({"xT": xTs[b], "w3": hv["w3"], "bqT": hv["bqT"],
                        "wot": hv["wot"]})
    return in_maps


def _numpy_fallback(x, attention_mask, Wq, bq, Wk, bk, Wv, bv, Wo, bo):
    out = np.empty((B, T, D), dtype=np.float32)
    neg = np.float32(np.finfo(np.float32).min)
    for b in range(B):
        xb = x[b]
        q = np.einsum("td,hed->hte", xb, Wq) + bq[:, None, :]
        k = np.einsum("td,hed->hte", xb, Wk) + bk[:, None, :]
        v = np.einsum("td,hed->hte", xb, Wv) + bv[:, None, :]
        s = np.einsum("hqe,hke->hqk", q, k).astype(np.float32) * np.float32(SCALE)
        causal = np.arange(T)[None, :] > np.arange(T)[:, None]
        s = np.where(causal[None], neg, s)
        keep = attention_mask[b].astype(bool)
        s = np.where(keep[None, None, :], s, neg)
        s = s - s.max(-1, keepdims=True)
        p = np.exp(s)
        p = p / p.sum(-1, keepdims=True)
        y = np.einsum("hqk,hke->hqe", p, v)
        y = np.transpose(y, (1, 0, 2)).reshape(T, D)
        out[b] = y @ Wo.T + bo
    return out


def kernel(x, attention_mask, Wq, bq, Wk, bk, Wv, bv, Wo, bo):
    x = np.asarray(x, dtype=np.float32)
    attention_mask = np.asarray(attention_mask)
    Wq, bq = np.asarray(Wq, np.float32), np.asarray(bq, np.float32)
    Wk, bk = np.asarray(Wk, np.float32), np.asarray(bk, np.float32)
    Wv, bv = np.asarray(Wv, np.float32), np.asarray(bv, np.float32)
    Wo, bo = np.asarray(Wo, np.float32), np.asarray(bo, np.float32)

    if not np.all(attention_mask == 1):
        return _numpy_fallback(x, attention_mask, Wq, bq, Wk, bk, Wv, bv, Wo, bo)

    from concourse.bass_utils import run_bass_kernel_spmd

    nc = _get_compiled()
    in_maps = _host_prep(x, Wq, bq, Wk, Wv, Wo)
    res = run_bass_kernel_spmd(nc, in_maps, core_ids=list(range(8)))

    # bv folds through softmax (rows sum to 1); bk is softmax-invariant
    bo_total = (bo + Wo @ bv.reshape(D)).astype(np.float32)

    out = np.zeros((B, T, D), dtype=np.float32)
    for c in range(8):
        partial = res.results[c]["outT"].reshape(D, T)  # [o, t]
        out[c // 2] += partial.T
    out += bo_total
    return out



# revision 8
# speedup vs baseline: 1.1839x; 1.1839x over previous
"""Trainium2 Bass kernel for multi-head causal self-attention.

Problem: B=4, T=1024, D=2048, H=16 heads, E=128 head_dim, fp32 I/O.
  q/k/v = per-head projections of x; scores = causal-masked softmax(q k^T / sqrt(E));
  y = probs @ v; out = concat-heads(y) @ Wo^T + bo.

Sharding: 8 cores = 4 batches x 2 head-halves. Core c handles batch c//2 and
heads (c%2)*8 .. (c%2)*8+7. Host sums the two half partials per batch and adds
the folded bias (bv folds through softmax into bo; bk is softmax-invariant).

Precision/throughput scheme:
  - Projections (A) and out-projection (C) run as fp8 DoubleRow matmuls at
    0.5 cycles/row with 2 contraction tiles packed per instruction. Full
    precision is recovered with a 3-term hi/lo decomposition:
        W@X ~= Whi@Xhi + Wlo@Xhi + Whi@Xlo
    where hi = e4m3(a), lo = a - hi (e5m2 for weights, whose lo falls below
    e4m3's subnormal range; e4m3 for activations). Cost is 0.75x of an
    fp32r/bf16 matmul at ~bf16-level accuracy.
  - x and all weights are hi/lo-split on the host. y (out-proj input) is
    split on-device: DVE writes y*1/r as fp32 scratch, ACT casts to e4m3 hi,
    DVE subtracts for the e4m3 lo.
  - Attention (B) runs in bf16: scores k-stationary/q-moving pre-transposed
    per key tile so exp writes the P@V moving operand straight to SBUF; row
    sums ride as an all-ones matmul next to y; 1/r is applied at y eviction.
    v is produced already [t,e]-transposed by Phase A (x-tiles stationary),
    so Phase B needs no PE transposes. exp absorbs the 1/sqrt(E) scale.
"""

import numpy as np

B, T, D, H = 4, 1024, 2048, 16
E = D // H            # 128
P = 128
ND = D // P           # 16 d-tiles
NP = ND // 2          # 8 d-tile pairs (DoubleRow)
NT = T // P           # 8 t-blocks / q-blocks / k-tiles
HL = H // 2           # 8 heads per core
NPC = HL // 2         # 4 i-tile pairs in out-proj
SCALE = 1.0 / np.sqrt(E)

_cache = {}


def _build():
    import concourse.bass as bass
    import concourse.mybir as mybir
    import concourse.tile as tile
    from concourse import bacc
    from concourse.bass import ts

    F32 = mybir.dt.float32
    BF16 = mybir.dt.bfloat16
    F8E4 = mybir.dt.float8e4
    F8E5 = mybir.dt.float8e5
    AF = mybir.ActivationFunctionType
    OP = mybir.AluOpType
    DR = mybir.MatmulPerfMode.DoubleRow

    nc = bacc.Bacc("TRN2", target_bir_lowering=False, debug=False)

    xh_d = nc.dram_tensor("xh", [P, ND, T], F8E4, kind="ExternalInput").ap()
    xl_d = nc.dram_tensor("xl", [P, ND, T], F8E4, kind="ExternalInput").ap()
    w3h_d = nc.dram_tensor("w3h", [HL, P, NP, 2, 3 * E], F8E4,
                           kind="ExternalInput").ap()
    w3l_d = nc.dram_tensor("w3l", [HL, P, NP, 2, 3 * E], F8E5,
                           kind="ExternalInput").ap()
    bqT_d = nc.dram_tensor("bqT", [P, HL], F32, kind="ExternalInput").ap()
    woth_d = nc.dram_tensor("woth", [ND, P, NPC, 2, P], F8E4,
                            kind="ExternalInput").ap()
    wotl_d = nc.dram_tensor("wotl", [ND, P, NPC, 2, P], F8E5,
                            kind="ExternalInput").ap()
    outT_d = nc.dram_tensor("outT", [ND, P, T], F32, kind="ExternalOutput").ap()

    QS, KS, VS = slice(0, E), slice(E, 2 * E), slice(2 * E, 3 * E)

    with tile.TileContext(nc) as tc:
        with (
            tc.tile_pool(name="const", bufs=1) as const,
            tc.tile_pool(name="qkv", bufs=1) as qkv,
            tc.tile_pool(name="small", bufs=4) as small,
        ):
            ones_f = const.tile([P, P], F32)
            nc.vector.memset(ones_f[:], 1.0)
            ones_b = const.tile([P, P], BF16)
            nc.vector.tensor_copy(ones_b[:], ones_f[:])
            bqT_t = const.tile([P, HL], F32)
            # all-zero fp8 operands: a full-bank "zero matmul" opens each psv
            # PSUM bank exactly once (one start per 2KB zero region; psv's
            # per-t-block groups then accumulate without further starts)
            zeros_t = const.tile([P, 2, 512], F8E4)
            nc.vector.memset(zeros_t[:], 0.0)

            qT = qkv.tile([P, HL, T], BF16)          # [e, head, t]
            kT = qkv.tile([P, HL, T], BF16)
            vA = qkv.tile([P, HL, NT, E], BF16)      # [t-in-block, head, tb, e]
            yh = qkv.tile([P, HL, T], F8E4)          # y hi  [e, head, t]
            yl = qkv.tile([P, HL, T], F8E4)          # y lo

            # ---------------- Phase A: q/k/v projections (fp8 DR) ----------
            with (
                tc.tile_pool(name="xp", bufs=1) as xp,
                tc.tile_pool(name="w3p", bufs=2) as w3p,
                tc.tile_pool(name="ps_a", bufs=4, space="PSUM") as ps_a,
            ):
                xh_t = xp.tile([P, ND, T], F8E4)
                xl_t = xp.tile([P, ND, T], F8E4)

                def w3_dma(hl, split=False):
                    w3h_t = w3p.tile([P, NP, 2, 3 * E], F8E4, tag="w3h",
                                     name="w3h_t")
                    w3l_t = w3p.tile([P, NP, 2, 3 * E], F8E5, tag="w3l",
                                     name="w3l_t")
                    if split:  # head 0: per-pair-pair groups so compute starts
                        for g in range(4):
                            nc.sync.dma_start(w3h_t[:, 2 * g:2 * g + 2],
                                              w3h_d[hl][:, 2 * g:2 * g + 2])
                    else:
                        nc.sync.dma_start(w3h_t[:], w3h_d[hl])
                    nc.sync.dma_start(w3l_t[:], w3l_d[hl])
                    return w3h_t, w3l_t

                # DMA order: first x pair + w3h head0 interleaved, then the
                # rest of xh, xl, w3l, so head0's main term starts asap.
                nc.sync.dma_start(xh_t[:, 0:2, :], xh_d[:, 0:2, :])
                w3_h0 = w3_dma(0, split=True)
                for dt in range(2, ND, 2):
                    nc.sync.dma_start(xh_t[:, dt:dt + 2, :], xh_d[:, dt:dt + 2, :])
                for dt in range(0, ND, 4):
                    nc.sync.dma_start(xl_t[:, dt:dt + 4, :], xl_d[:, dt:dt + 4, :])
                nc.sync.dma_start(bqT_t[:], bqT_d)
                # preload the ACT Exp table off the critical path
                dummy = small.tile([P, 1], F32, tag="racc", name="dummy")
                nc.scalar.activation(dummy[:], bqT_t[:, 0:1], AF.Exp)

                for hl in range(HL):
                    w3h_t, w3l_t = w3_h0 if hl == 0 else w3_nx[0]
                    psq = ps_a.tile([P, T], F32, tag="a")
                    psk = ps_a.tile([P, T], F32, tag="a")
                    psv = ps_a.tile([P, NT, E], F32, tag="a")
                    for bank in range(2):
                        nc.tensor.matmul(
                            psv[:, 4 * bank:4 * bank + 4, :],
                            zeros_t[:, :, 0:P], zeros_t[:],
                            start=True, stop=False, perf_mode=DR,
                            skip_group_check=True)

                    # terms: (stationary-w, moving-x) for q/k;
                    #        (stationary-x, moving-w) for v
                    # head 0 defers the w3l term until its DMA lands.
                    terms = [(w3h_t, xh_t), (w3l_t, xh_t), (w3h_t, xl_t)]
                    if hl == 0:
                        terms = [terms[0], terms[2], terms[1]]

                    for ti, (wt, xt) in enumerate(terms):
                        st, sp = ti == 0, ti == 2
                        for pr in range(NP):
                            s0, sN = st and pr == 0, sp and pr == NP - 1
                            xpair = xt[:, 2 * pr:2 * pr + 2, :]
                            for c in range(2):
                                nc.tensor.matmul(
                                    psq[:, ts(c, 512)], wt[:, pr, :, QS],
                                    xpair[:, :, ts(c, 512)], start=s0, stop=sN,
                                    perf_mode=DR, skip_group_check=True)
                                nc.tensor.matmul(
                                    psk[:, ts(c, 512)], wt[:, pr, :, KS],
                                    xpair[:, :, ts(c, 512)], start=s0, stop=sN,
                                    perf_mode=DR, skip_group_check=True)
                            for tb in range(NT):
                                nc.tensor.matmul(
                                    psv[:, tb, :], xpair[:, :, ts(tb, P)],
                                    wt[:, pr, :, VS], start=False,
                                    stop=sN and tb % 4 == 3,
                                    perf_mode=DR, skip_group_check=True)

                    # prefetch next head's weights while this head computes
                    if hl + 1 < HL:
                        w3_nx = [w3_dma(hl + 1)]

                    # evict: q gets +bq on DVE; k on ACT; v on DVE (bf16 out)
                    nc.vector.tensor_scalar(
                        qT[:, hl, :], psq[:], bqT_t[:, hl:hl + 1], None,
                        op0=OP.add)
                    nc.scalar.activation(kT[:, hl, :], psk[:], AF.Copy)
                    nc.vector.tensor_copy(vA[:, hl], psv[:])

            # ---------------- Phases B+C scope ----------------
            with (
                tc.tile_pool(name="wop", bufs=3) as wop,
            ):
                def wot_dma(ob):
                    woth_t = wop.tile([P, NPC, 2, P], F8E4, tag="woh",
                                      name="woth_t")
                    wotl_t = wop.tile([P, NPC, 2, P], F8E5, tag="wol",
                                      name="wotl_t")
                    nc.sync.dma_start(woth_t[:], woth_d[ob])
                    nc.sync.dma_start(wotl_t[:], wotl_d[ob])
                    return woth_t, wotl_t

                wot_pre = [wot_dma(ob) for ob in range(2)]

                # -------- Phase B: attention per head (bf16) --------
                with (
                    tc.tile_pool(name="etp", bufs=1) as etp,
                    tc.tile_pool(name="rbp", bufs=2) as rbp,
                    tc.tile_pool(name="yfp", bufs=2) as yfp,
                    tc.tile_pool(name="ps_s", bufs=4, space="PSUM") as ps_s,
                    tc.tile_pool(name="ps_ar", bufs=2, space="PSUM") as ps_ar,
                ):
                    tail_work = [None]
                    for hl in range(HL):
                        ET = etp.tile([P, NT, T], BF16, tag="ET")  # [k, kt, q]
                        rb = rbp.tile([P, T], F32, tag="rb")       # 1/r rows

                        def emit_ST(j, ET=ET, hl=hl):
                            kblk = kT[:, hl, ts(j, P)]
                            if j < 4:
                                wA = 512 - j * P
                                sA = ps_s.tile([P, 512], F32, tag="s", name="sA")
                                nc.tensor.matmul(sA[:, 0:wA], kblk,
                                                 qT[:, hl, j * P:512],
                                                 start=True, stop=True)
                                sB = ps_s.tile([P, 512], F32, tag="s", name="sB")
                                nc.tensor.matmul(sB[:], kblk, qT[:, hl, 512:T],
                                                 start=True, stop=True)
                                nc.scalar.activation(ET[:, j, j * P:(j + 1) * P],
                                                     sA[:, 0:P], AF.Exp,
                                                     scale=float(SCALE))
                                nc.gpsimd.affine_select(
                                    out=ET[:, j, j * P:(j + 1) * P],
                                    in_=ET[:, j, j * P:(j + 1) * P],
                                    compare_op=OP.is_ge, fill=0.0,
                                    base=0, pattern=[[1, P]], channel_multiplier=-1,
                                )
                                if wA > P:
                                    nc.scalar.activation(ET[:, j, (j + 1) * P:512],
                                                         sA[:, P:wA], AF.Exp,
                                                         scale=float(SCALE))
                                nc.scalar.activation(ET[:, j, 512:T], sB[:],
                                                     AF.Exp, scale=float(SCALE))
                            else:
                                lo = j * P - 512
                                sB = ps_s.tile([P, 512], F32, tag="s", name="sB")
                                nc.tensor.matmul(sB[:, lo:512], kblk,
                                                 qT[:, hl, j * P:T],
                                                 start=True, stop=True)
                                nc.scalar.activation(ET[:, j, j * P:(j + 1) * P],
                                                     sB[:, lo:lo + P], AF.Exp,
                                                     scale=float(SCALE))
                                nc.gpsimd.affine_select(
                                    out=ET[:, j, j * P:(j + 1) * P],
                                    in_=ET[:, j, j * P:(j + 1) * P],
                                    compare_op=OP.is_ge, fill=0.0,
                                    base=0, pattern=[[1, P]], channel_multiplier=-1,
                                )
                                if j < NT - 1:
                                    nc.scalar.activation(ET[:, j, (j + 1) * P:T],
                                                         sB[:, lo + P:512],
                                                         AF.Exp, scale=float(SCALE))

                        # y and r accumulate per 512-wide q-chunk
                        y0 = ps_ar.tile([P, 512], F32, tag="y", name="y0")
                        r0 = ps_ar.tile([P, 512], F32, tag="r", name="r0")
                        y1 = ps_ar.tile([P, 512], F32, tag="y", name="y1")
                        r1 = ps_ar.tile([P, 512], F32, tag="r", name="r1")

                        def emit_AVR(jq, y0=y0, r0=r0, y1=y1, r1=r1, ET=ET, hl=hl):
                            vblk = vA[:, hl, jq, :]
                            if jq <= 3:
                                lo = jq * P
                                st, sp = jq == 0, jq == 3
                                nc.tensor.matmul(y0[:, lo:512], vblk,
                                                 ET[:, jq, lo:512], start=st, stop=sp,
                                                 skip_group_check=True)
                                nc.tensor.matmul(r0[:, lo:512], ones_b[:],
                                                 ET[:, jq, lo:512], start=st, stop=sp,
                                                 skip_group_check=True)
                            lo = max(jq * P, 512)
                            st, sp = jq == 0, jq == NT - 1
                            nc.tensor.matmul(y1[:, lo - 512:512], vblk,
                                             ET[:, jq, lo:T], start=st, stop=sp,
                                             skip_group_check=True)
                            nc.tensor.matmul(r1[:, lo - 512:512], ones_b[:],
                                             ET[:, jq, lo:T], start=st, stop=sp,
                                             skip_group_check=True)

                        def emit_ynorm(c, y_ps, hl=hl, rb=rb):
                            # yf = y * (1/r) in f32; hi = e4m3(yf); lo = yf - hi
                            yf = yfp.tile([P, 512], F32, tag="yf", name="yf")
                            nc.vector.reciprocal(rb[:, ts(c, 512)], y_ps[1][:])
                            nc.vector.tensor_mul(yf[:], y_ps[0][:],
                                                 rb[:, ts(c, 512)])
                            nc.scalar.activation(yh[:, hl, ts(c, 512)], yf[:],
                                                 AF.Copy)
                            nc.vector.tensor_tensor(
                                yl[:, hl, ts(c, 512)], yf[:],
                                yh[:, hl, ts(c, 512)], op=OP.subtract)

                        for j in range(NT):
                            emit_ST(j)
                            if j == 1 and tail_work[0] is not None:
                                tail_work[0]()
                                tail_work[0] = None
                            if j >= 2:
                                emit_AVR(j - 2)
                            if j == 5:
                                emit_ynorm(0, (y0, r0))
                        emit_AVR(NT - 2)
                        emit_AVR(NT - 1)

                        def tail(y1=y1, r1=r1, hl=hl):
                            emit_ynorm(1, (y1, r1), hl=hl)

                        if hl + 1 < HL:
                            tail_work[0] = tail
                        else:
                            tail()

                # -------- Phase C: partial out-projection (fp8 DR) --------
                with (
                    tc.tile_pool(name="osb", bufs=2) as osb,
                    tc.tile_pool(name="ps_o", bufs=2, space="PSUM") as ps_o,
                ):
                    for ob in range(ND):
                        woth_t, wotl_t = wot_pre[ob] if ob < 2 else wot_dma(ob)
                        o_ps = ps_o.tile([P, T], F32, tag="o")
                        for c in range(2):
                            n = 0
                            for wt, yt in ((woth_t, yh), (wotl_t, yh),
                                           (woth_t, yl)):
                                for pr in range(NPC):
                                    nc.tensor.matmul(
                                        o_ps[:, ts(c, 512)], wt[:, pr],
                                        yt[:, 2 * pr:2 * pr + 2, ts(c, 512)],
                                        start=n == 0, stop=n == 3 * NPC - 1,
                                        perf_mode=DR, skip_group_check=True)
                                    n += 1
                        out_sb = osb.tile([P, T], F32, tag="osb")
                        nc.scalar.activation(out_sb[:], o_ps[:], AF.Copy)
                        nc.sync.dma_start(outT_d[ob], out_sb[:])

    nc.compile()
    return nc


def _get_compiled():
    if "nc" not in _cache:
        _cache["nc"] = _build()
    return _cache["nc"]


def _hilo(a, lo_dt):
    import ml_dtypes
    hi = np.ascontiguousarray(a).astype(ml_dtypes.float8_e4m3)
    lo = (a - hi.astype(np.float32)).astype(lo_dt)
    return hi, lo


def _host_prep(x, Wq, bq, Wk, Wv, Wo):
    """Build per-core input maps (hi/lo fp8 splits + DR pair packing)."""
    import ml_dtypes
    E4, E5 = ml_dtypes.float8_e4m3, ml_dtypes.float8_e5m2
    in_maps = []
    xs = []
    for b in range(B):
        # [P, ND, T]: row p holds d-tile-major slices, matching the SBUF tile
        xT = np.ascontiguousarray(
            x[b].T.reshape(ND, P, T).transpose(1, 0, 2))
        xs.append(_hilo(xT, E4))
    halves = []
    for half in range(2):
        hs = slice(half * HL, (half + 1) * HL)
        # w3 packed [HL, P, NP, 2, 3E]: slot s of pair pr = d-tile 2pr+s
        w3 = np.empty((HL, P, NP, 2, 3 * E), dtype=np.float32)
        for hl, h in enumerate(range(half * HL, (half + 1) * HL)):
            for j, W in enumerate((Wq[h], Wk[h], Wv[h])):
                wt = W.T.reshape(NP, 2, P, E)          # [pr, s, p(d), e]
                w3[hl, :, :, :, j * E:(j + 1) * E] = wt.transpose(2, 0, 1, 3)
        w3h, w3l = _hilo(w3, E5)
        bqT = np.ascontiguousarray(bq[hs].T)           # [E, HL]
        # wot [ND, P, NPC, 2, P]: [ob, i-in-tile, pr, s, o], i-tile = 2pr+s
        WoT_span = Wo.T[half * 1024:(half + 1) * 1024]  # [1024, D]
        wot = np.ascontiguousarray(
            WoT_span.reshape(NPC, 2, P, ND, P).transpose(3, 2, 0, 1, 4))
        woth, wotl = _hilo(wot, E5)
        halves.append({"w3h": w3h, "w3l": w3l, "bqT": bqT,
                       "woth": woth, "wotl": wotl})
    for c in range(8):
        b, half = c // 2, c % 2
        hv = halves[half]
        in_maps.append({"xh": xs[b][0], "xl": xs[b][1], **hv})
    return in_maps


def _numpy_fallback(x, attention_mask, Wq, bq, Wk, bk, Wv, bv, Wo, bo):
    out = np.empty((B, T, D), dtype=np.float32)
    neg = np.float32(np.finfo(np.float32).min)
    for b in range(B):
        xb = x[b]
        q = np.einsum("td,hed->hte", xb, Wq) + bq[:, None, :]
        k = np.einsum("td,hed->hte", xb, Wk) + bk[:, None, :]
        v = np.einsum("td,hed->hte", xb, Wv) + bv[:, None, :]
        s = np.einsum("hqe,hke->hqk", q, k).astype(np.float32) * np.float32(SCALE)
        causal = np.arange(T)[None, :] > np.arange(T)[:, None]
        s = np.where(causal[None], neg, s)
        keep = attention_mask[b].astype(bool)
        s = np.where(keep[None, None, :], s, neg)
        s = s - s.max(-1, keepdims=True)
        p = np.exp(s)
        p = p / p.sum(-1, keepdims=True)
        y = np.einsum("hqk,hke->hqe", p, v)
        y = np.transpose(y, (1, 0, 2)).reshape(T, D)
        out[b] = y @ Wo.T + bo
    return out


def kernel(x, attention_mask, Wq, bq, Wk, bk, Wv, bv, Wo, bo):
    x = np.asarray(x, dtype=np.float32)
    attention_mask = np.asarray(attention_mask)
    Wq, bq = np.asarray(Wq, np.float32), np.asarray(bq, np.float32)
    Wk, bk = np.asarray(Wk, np.float32), np.asarray(bk, np.float32)
    Wv, bv = np.asarray(Wv, np.float32), np.asarray(bv, np.float32)
    Wo, bo = np.asarray(Wo, np.float32), np.asarray(bo, np.float32)

    if not np.all(attention_mask == 1):
        return _numpy_fallback(x, attention_mask, Wq, bq, Wk, bk, Wv, bv, Wo, bo)

    from concourse.bass_utils import run_bass_kernel_spmd

    nc = _get_compiled()
    in_maps = _host_prep(x, Wq, bq, Wk, Wv, Wo)
    res = run_bass_kernel_spmd(nc, in_maps, core_ids=list(range(8)))

    # bv folds through softmax (rows sum to 1); bk is softmax-invariant
    bo_total = (bo + Wo @ bv.reshape(D)).astype(np.float32)

    out = np.zeros((B, T, D), dtype=np.float32)
    for c in range(8):
        partial = res.results[c]["outT"].reshape(D, T)  # [o, t]
        out[c // 2] += partial.T
    out += bo_total
    return out


# revision 15
# speedup vs baseline: 1.2511x; 1.0568x over previous
"""Trainium2 Bass kernel for multi-head causal self-attention.

Problem: B=4, T=1024, D=2048, H=16 heads, E=128 head_dim, fp32 I/O.
  q/k/v = per-head projections of x; scores = causal-masked softmax(q k^T / sqrt(E));
  y = probs @ v; out = concat-heads(y) @ Wo^T + bo.

Sharding: 8 cores = 4 batches x 2 head-halves. Core c handles batch c//2 and
heads (c%2)*8 .. (c%2)*8+7. Host sums the two half partials per batch and adds
the folded bias (bv folds through softmax into bo; bk is softmax-invariant).

Precision/throughput scheme:
  - Projections (A) and out-projection (C) run as fp8 DoubleRow matmuls at
    0.5 cycles/row with 2 contraction tiles packed per instruction. Full
    precision is recovered with a 3-term hi/lo decomposition:
        W@X ~= Whi@Xhi + Wlo@Xhi + Whi@Xlo
    where hi = e4m3(a), lo = a - hi (e5m2 for weights, whose lo falls below
    e4m3's subnormal range; e4m3 for activations). Cost is 0.75x of an
    fp32r/bf16 matmul at ~bf16-level accuracy.
  - x and all weights are hi/lo-split on the host. y (out-proj input) is
    split on-device: DVE writes y*1/r as fp32 scratch, ACT casts to e4m3 hi,
    DVE subtracts for the e4m3 lo.
  - Attention (B) runs in bf16: scores k-stationary/q-moving pre-transposed
    per key tile so exp writes the P@V moving operand straight to SBUF; row
    sums ride as an all-ones matmul next to y; 1/r is applied at y eviction.
    v is produced already [t,e]-transposed by Phase A (x-tiles stationary),
    so Phase B needs no PE transposes. exp absorbs the 1/sqrt(E) scale.
"""

import numpy as np

B, T, D, H = 4, 1024, 2048, 16
E = D // H            # 128
P = 128
ND = D // P           # 16 d-tiles
NP = ND // 2          # 8 d-tile pairs (DoubleRow)
NT = T // P           # 8 t-blocks / q-blocks / k-tiles
HL = H // 2           # 8 heads per core
NPC = HL // 2         # 4 i-tile pairs in out-proj
SCALE = 1.0 / np.sqrt(E)

_cache = {}


def _build():
    import concourse.bass as bass
    import concourse.mybir as mybir
    import concourse.tile as tile
    from concourse import bacc
    from concourse.bass import ts

    F32 = mybir.dt.float32
    BF16 = mybir.dt.bfloat16
    F8E4 = mybir.dt.float8e4
    F8E5 = mybir.dt.float8e5
    AF = mybir.ActivationFunctionType
    OP = mybir.AluOpType
    DR = mybir.MatmulPerfMode.DoubleRow

    nc = bacc.Bacc("TRN2", target_bir_lowering=False, debug=False)

    xh_d = nc.dram_tensor("xh", [P, ND, T], F8E4, kind="ExternalInput").ap()
    xl_d = nc.dram_tensor("xl", [P, ND, T], F8E4, kind="ExternalInput").ap()
    w3h_d = nc.dram_tensor("w3h", [HL, P, NP, 2, 3 * E], F8E4,
                           kind="ExternalInput").ap()
    w3l_d = nc.dram_tensor("w3l", [HL, P, NP, 2, 3 * E], F8E5,
                           kind="ExternalInput").ap()
    bqT_d = nc.dram_tensor("bqT", [P, HL], F32, kind="ExternalInput").ap()
    woth_d = nc.dram_tensor("woth", [ND, P, NPC, 2, P], F8E4,
                            kind="ExternalInput").ap()
    wotl_d = nc.dram_tensor("wotl", [ND, P, NPC, 2, P], F8E5,
                            kind="ExternalInput").ap()
    outT_d = nc.dram_tensor("outT", [ND, P, T], F32, kind="ExternalOutput").ap()

    QS, KS, VS = slice(0, E), slice(E, 2 * E), slice(2 * E, 3 * E)

    with tile.TileContext(nc) as tc:
        with (
            tc.tile_pool(name="const", bufs=1) as const,
            tc.tile_pool(name="qkv", bufs=1) as qkv,
            tc.tile_pool(name="small", bufs=4) as small,
        ):
            ones_f = const.tile([P, P], F32)
            nc.vector.memset(ones_f[:], 1.0)
            ones_b = const.tile([P, P], BF16)
            nc.vector.tensor_copy(ones_b[:], ones_f[:])
            bqT_t = const.tile([P, HL], F32)
            # all-zero fp8 operands: a full-bank "zero matmul" opens each psv
            # PSUM bank exactly once (one start per 2KB zero region; psv's
            # per-t-block groups then accumulate without further starts)
            zeros_t = const.tile([P, 2, 512], F8E4)
            nc.vector.memset(zeros_t[:], 0.0)

            qT = qkv.tile([P, HL, T], BF16)          # [e, head, t]
            kT = qkv.tile([P, HL, T], BF16)
            vA = qkv.tile([P, HL, NT, E], BF16)      # [t-in-block, head, tb, e]
            yh = qkv.tile([P, HL, T], F8E4)          # y hi  [e, head, t]
            yl = qkv.tile([P, HL, T], F8E4)          # y lo

            # ---------------- Phase A: q/k/v projections (fp8 DR) ----------
            with (
                tc.tile_pool(name="xp", bufs=1) as xp,
                tc.tile_pool(name="w3p", bufs=2) as w3p,
                tc.tile_pool(name="ps_a", bufs=4, space="PSUM") as ps_a,
            ):
                xh_t = xp.tile([P, ND, T], F8E4)
                xl_t = xp.tile([P, ND, T], F8E4)

                def w3_dma(hl, split=False):
                    w3h_t = w3p.tile([P, NP, 2, 3 * E], F8E4, tag="w3h",
                                     name="w3h_t")
                    w3l_t = w3p.tile([P, NP, 2, 3 * E], F8E5, tag="w3l",
                                     name="w3l_t")
                    if split:  # head 0: per-pair-pair groups so compute
                        # starts early; caller issues the w3l DMA later
                        for g in range(4):
                            nc.sync.dma_start(w3h_t[:, 2 * g:2 * g + 2],
                                              w3h_d[hl][:, 2 * g:2 * g + 2])
                    else:
                        nc.sync.dma_start(w3h_t[:], w3h_d[hl])
                        nc.sync.dma_start(w3l_t[:], w3l_d[hl])
                    return w3h_t, w3l_t

                # DMA order: xh pairs + w3h(h0) interleaved (main term streams),
                # then w3l(h0) so wcorr runs full-speed while xl streams last.
                nc.sync.dma_start(xh_t[:, 0:2, :], xh_d[:, 0:2, :])
                w3_h0 = w3_dma(0, split=True)
                for dt in range(2, ND, 2):
                    nc.sync.dma_start(xh_t[:, dt:dt + 2, :], xh_d[:, dt:dt + 2, :])
                nc.sync.dma_start(w3_h0[1][:], w3l_d[0])
                for dt in range(0, ND, 4):
                    nc.sync.dma_start(xl_t[:, dt:dt + 4, :], xl_d[:, dt:dt + 4, :])
                nc.sync.dma_start(bqT_t[:], bqT_d)
                # preload the ACT Exp table off the critical path
                dummy = small.tile([P, 1], F32, tag="racc", name="dummy")
                nc.scalar.activation(dummy[:], bqT_t[:, 0:1], AF.Exp)

                for hl in range(HL):
                    w3h_t, w3l_t = w3_h0 if hl == 0 else w3_nx[0]
                    psq = ps_a.tile([P, T], F32, tag="a")
                    psk = ps_a.tile([P, T], F32, tag="a")
                    psv = ps_a.tile([P, NT, E], F32, tag="a")
                    for bank in range(2):
                        nc.tensor.matmul(
                            psv[:, 4 * bank:4 * bank + 4, :],
                            zeros_t[:, :, 0:P], zeros_t[:],
                            start=True, stop=False, perf_mode=DR,
                            skip_group_check=True)

                    # terms: (stationary-w, moving-x) for q/k;
                    #        (stationary-x, moving-w) for v
                    terms = [(w3h_t, xh_t), (w3l_t, xh_t), (w3h_t, xl_t)]

                    for ti, (wt, xt) in enumerate(terms):
                        st, sp = ti == 0, ti == 2
                        for pr in range(NP):
                            s0, sN = st and pr == 0, sp and pr == NP - 1
                            xpair = xt[:, 2 * pr:2 * pr + 2, :]
                            for c in range(2):
                                nc.tensor.matmul(
                                    psq[:, ts(c, 512)], wt[:, pr, :, QS],
                                    xpair[:, :, ts(c, 512)], start=s0, stop=sN,
                                    perf_mode=DR, skip_group_check=True)
                                nc.tensor.matmul(
                                    psk[:, ts(c, 512)], wt[:, pr, :, KS],
                                    xpair[:, :, ts(c, 512)], start=s0, stop=sN,
                                    perf_mode=DR, skip_group_check=True)
                            for tb in range(NT):
                                nc.tensor.matmul(
                                    psv[:, tb, :], xpair[:, :, ts(tb, P)],
                                    wt[:, pr, :, VS], start=False,
                                    stop=sN and tb % 4 == 3,
                                    perf_mode=DR, skip_group_check=True)

                    # prefetch next head's weights while this head computes
                    if hl + 1 < HL:
                        w3_nx = [w3_dma(hl + 1)]

                    # evict: q gets +bq on DVE; k on ACT; v on DVE (bf16 out)
                    nc.vector.tensor_scalar(
                        qT[:, hl, :], psq[:], bqT_t[:, hl:hl + 1], None,
                        op0=OP.add)
                    nc.scalar.activation(kT[:, hl, :], psk[:], AF.Copy)
                    nc.vector.tensor_copy(vA[:, hl], psv[:])

            # ---------------- Phases B+C scope ----------------
            with (
                tc.tile_pool(name="wop", bufs=3) as wop,
            ):
                def wot_dma(ob):
                    woth_t = wop.tile([P, NPC, 2, P], F8E4, tag="woh",
                                      name="woth_t")
                    wotl_t = wop.tile([P, NPC, 2, P], F8E5, tag="wol",
                                      name="wotl_t")
                    nc.sync.dma_start(woth_t[:], woth_d[ob])
                    nc.sync.dma_start(wotl_t[:], wotl_d[ob])
                    return woth_t, wotl_t

                wot_pre = [wot_dma(ob) for ob in range(2)]

                # -------- Phase B: attention per head (bf16) --------
                with (
                    tc.tile_pool(name="etp", bufs=1) as etp,
                    tc.tile_pool(name="rbp", bufs=2) as rbp,
                    tc.tile_pool(name="yfp", bufs=2) as yfp,
                    tc.tile_pool(name="ps_s", bufs=2, space="PSUM") as ps_s,
                    tc.tile_pool(name="ps_ar", bufs=2, space="PSUM") as ps_ar,
                ):
                    tail_work = [None]
                    for hl in range(HL):
                        ET = etp.tile([P, NT, T], BF16, tag="ET")  # [k, kt, q]
                        rb = rbp.tile([P, T], F32, tag="rb")       # 1/r rows

                        def emit_ST(j, ET=ET, hl=hl):
                            # scores for k-tile j land in one [P, T] psum so
                            # a single exp covers the whole causal row range
                            kblk = kT[:, hl, ts(j, P)]
                            s_ps = ps_s.tile([P, T], F32, tag="s", name="s_ps")
                            if j < 4:
                                nc.tensor.matmul(s_ps[:, j * P:512], kblk,
                                                 qT[:, hl, j * P:512],
                                                 start=True, stop=True)
                                nc.tensor.matmul(s_ps[:, 512:T], kblk,
                                                 qT[:, hl, 512:T],
                                                 start=True, stop=True)
                            else:
                                nc.tensor.matmul(s_ps[:, j * P:T], kblk,
                                                 qT[:, hl, j * P:T],
                                                 start=True, stop=True)
                            nc.scalar.activation(ET[:, j, j * P:T],
                                                 s_ps[:, j * P:T], AF.Exp,
                                                 scale=float(SCALE))
                            nc.gpsimd.affine_select(
                                out=ET[:, j, j * P:(j + 1) * P],
                                in_=ET[:, j, j * P:(j + 1) * P],
                                compare_op=OP.is_ge, fill=0.0,
                                base=0, pattern=[[1, P]], channel_multiplier=-1,
                            )

                        # y and r accumulate per 512-wide q-chunk
                        y0 = ps_ar.tile([P, 512], F32, tag="y", name="y0")
                        r0 = ps_ar.tile([P, 512], F32, tag="r", name="r0")
                        y1 = ps_ar.tile([P, 512], F32, tag="y", name="y1")
                        r1 = ps_ar.tile([P, 512], F32, tag="r", name="r1")

                        def emit_AVR(jq, y0=y0, r0=r0, y1=y1, r1=r1, ET=ET, hl=hl):
                            vblk = vA[:, hl, jq, :]
                            if jq <= 3:
                                lo = jq * P
                                st, sp = jq == 0, jq == 3
                                nc.tensor.matmul(y0[:, lo:512], vblk,
                                                 ET[:, jq, lo:512], start=st, stop=sp,
                                                 skip_group_check=True)
                                nc.tensor.matmul(r0[:, lo:512], ones_b[:],
                                                 ET[:, jq, lo:512], start=st, stop=sp,
                                                 skip_group_check=True)
                            lo = max(jq * P, 512)
                            st, sp = jq == 0, jq == NT - 1
                            nc.tensor.matmul(y1[:, lo - 512:512], vblk,
                                             ET[:, jq, lo:T], start=st, stop=sp,
                                             skip_group_check=True)
                            nc.tensor.matmul(r1[:, lo - 512:512], ones_b[:],
                                             ET[:, jq, lo:T], start=st, stop=sp,
                                             skip_group_check=True)

                        def emit_ynorm(c, y_ps, hl=hl, rb=rb):
                            # yf = y * (1/r) in f32; hi = e4m3(yf); lo = yf - hi
                            # (all on DVE: ACT is the busier engine in B)
                            yf = yfp.tile([P, 512], F32, tag="yf", name="yf")
                            nc.vector.reciprocal(rb[:, ts(c, 512)], y_ps[1][:])
                            nc.vector.tensor_mul(yf[:], y_ps[0][:],
                                                 rb[:, ts(c, 512)])
                            nc.vector.tensor_copy(yh[:, hl, ts(c, 512)], yf[:])
                            nc.vector.tensor_tensor(
                                yl[:, hl, ts(c, 512)], yf[:],
                                yh[:, hl, ts(c, 512)], op=OP.subtract)

                        for j in range(NT):
                            emit_ST(j)
                            if j == 1 and tail_work[0] is not None:
                                tail_work[0]()
                                tail_work[0] = None
                            if j >= 2:
                                emit_AVR(j - 2)
                            if j == 5:
                                emit_ynorm(0, (y0, r0))
                        emit_AVR(NT - 2)
                        emit_AVR(NT - 1)

                        def tail(y1=y1, r1=r1, hl=hl):
                            emit_ynorm(1, (y1, r1), hl=hl)

                        if hl + 1 < HL:
                            tail_work[0] = tail
                        else:
                            tail()

                # -------- Phase C: partial out-projection (fp8 DR) --------
                with (
                    tc.tile_pool(name="osb", bufs=2) as osb,
                    tc.tile_pool(name="ps_o", bufs=2, space="PSUM") as ps_o,
                ):
                    for ob in range(ND):
                        woth_t, wotl_t = wot_pre[ob] if ob < 2 else wot_dma(ob)
                        o_ps = ps_o.tile([P, T], F32, tag="o")
                        out_sb = osb.tile([P, T], F32, tag="osb")
                        for c in range(2):
                            n = 0
                            for wt, yt in ((woth_t, yh), (wotl_t, yh),
                                           (woth_t, yl)):
                                for pr in range(NPC):
                                    nc.tensor.matmul(
                                        o_ps[:, ts(c, 512)], wt[:, pr],
                                        yt[:, 2 * pr:2 * pr + 2, ts(c, 512)],
                                        start=n == 0, stop=n == 3 * NPC - 1,
                                        perf_mode=DR, skip_group_check=True)
                                    n += 1
                            # per-chunk evict + DMA so the store pipeline
                            # overlaps the second chunk's matmuls
                            nc.scalar.activation(out_sb[:, ts(c, 512)],
                                                 o_ps[:, ts(c, 512)], AF.Copy)
                            nc.sync.dma_start(outT_d[ob][:, ts(c, 512)],
                                              out_sb[:, ts(c, 512)])

    nc.compile()
    return nc


def _get_compiled():
    if "nc" not in _cache:
        _cache["nc"] = _build()
    return _cache["nc"]


def _hilo(a, lo_dt):
    import ml_dtypes
    hi = np.ascontiguousarray(a).astype(ml_dtypes.float8_e4m3)
    lo = (a - hi.astype(np.float32)).astype(lo_dt)
    return hi, lo


def _host_prep(x, Wq, bq, Wk, Wv, Wo):
    """Build per-core input maps (hi/lo fp8 splits + DR pair packing)."""
    import ml_dtypes
    E4, E5 = ml_dtypes.float8_e4m3, ml_dtypes.float8_e5m2
    in_maps = []
    xs = []
    for b in range(B):
        # [P, ND, T]: row p holds d-tile-major slices, matching the SBUF tile
        xT = np.ascontiguousarray(
            x[b].T.reshape(ND, P, T).transpose(1, 0, 2))
        xs.append(_hilo(xT, E4))
    halves = []
    for half in range(2):
        hs = slice(half * HL, (half + 1) * HL)
        # w3 packed [HL, P, NP, 2, 3E]: slot s of pair pr = d-tile 2pr+s
        w3 = np.empty((HL, P, NP, 2, 3 * E), dtype=np.float32)
        for hl, h in enumerate(range(half * HL, (half + 1) * HL)):
            for j, W in enumerate((Wq[h], Wk[h], Wv[h])):
                wt = W.T.reshape(NP, 2, P, E)          # [pr, s, p(d), e]
                w3[hl, :, :, :, j * E:(j + 1) * E] = wt.transpose(2, 0, 1, 3)
        w3h, w3l = _hilo(w3, E5)
        bqT = np.ascontiguousarray(bq[hs].T)           # [E, HL]
        # wot [ND, P, NPC, 2, P]: [ob, i-in-tile, pr, s, o], i-tile = 2pr+s
        WoT_span = Wo.T[half * 1024:(half + 1) * 1024]  # [1024, D]
        wot = np.ascontiguousarray(
            WoT_span.reshape(NPC, 2, P, ND, P).transpose(3, 2, 0, 1, 4))
        woth, wotl = _hilo(wot, E5)
        halves.append({"w3h": w3h, "w3l": w3l, "bqT": bqT,
                       "woth": woth, "wotl": wotl})
    for c in range(8):
        b, half = c // 2, c % 2
        hv = halves[half]
        in_maps.append({"xh": xs[b][0], "xl": xs[b][1], **hv})
    return in_maps


def _numpy_fallback(x, attention_mask, Wq, bq, Wk, bk, Wv, bv, Wo, bo):
    out = np.empty((B, T, D), dtype=np.float32)
    neg = np.float32(np.finfo(np.float32).min)
    for b in range(B):
        xb = x[b]
        q = np.einsum("td,hed->hte", xb, Wq) + bq[:, None, :]
        k = np.einsum("td,hed->hte", xb, Wk) + bk[:, None, :]
        v = np.einsum("td,hed->hte", xb, Wv) + bv[:, None, :]
        s = np.einsum("hqe,hke->hqk", q, k).astype(np.float32) * np.float32(SCALE)
        causal = np.arange(T)[None, :] > np.arange(T)[:, None]
        s = np.where(causal[None], neg, s)
        keep = attention_mask[b].astype(bool)
        s = np.where(keep[None, None, :], s, neg)
        s = s - s.max(-1, keepdims=True)
        p = np.exp(s)
        p = p / p.sum(-1, keepdims=True)
        y = np.einsum("hqk,hke->hqe", p, v)
        y = np.transpose(y, (1, 0, 2)).reshape(T, D)
        out[b] = y @ Wo.T + bo
    return out


def kernel(x, attention_mask, Wq, bq, Wk, bk, Wv, bv, Wo, bo):
    x = np.asarray(x, dtype=np.float32)
    attention_mask = np.asarray(attention_mask)
    Wq, bq = np.asarray(Wq, np.float32), np.asarray(bq, np.float32)
    Wk, bk = np.asarray(Wk, np.float32), np.asarray(bk, np.float32)
    Wv, bv = np.asarray(Wv, np.float32), np.asarray(bv, np.float32)
    Wo, bo = np.asarray(Wo, np.float32), np.asarray(bo, np.float32)

    if not np.all(attention_mask == 1):
        return _numpy_fallback(x, attention_mask, Wq, bq, Wk, bk, Wv, bv, Wo, bo)

    from concourse.bass_utils import run_bass_kernel_spmd

    nc = _get_compiled()
    in_maps = _host_prep(x, Wq, bq, Wk, Wv, Wo)
    res = run_bass_kernel_spmd(nc, in_maps, core_ids=list(range(8)))

    # bv folds through softmax (rows sum to 1); bk is softmax-invariant
    bo_total = (bo + Wo @ bv.reshape(D)).astype(np.float32)

    out = np.zeros((B, T, D), dtype=np.float32)
    for c in range(8):
        partial = res.results[c]["outT"].reshape(D, T)  # [o, t]
        out[c // 2] += partial.T
    out += bo_total
    return out


# revision 18
# speedup vs baseline: 1.2607x; 1.0076x over previous
"""Trainium2 Bass kernel for multi-head causal self-attention.

Problem: B=4, T=1024, D=2048, H=16 heads, E=128 head_dim, fp32 I/O.
  q/k/v = per-head projections of x; scores = causal-masked softmax(q k^T / sqrt(E));
  y = probs @ v; out = concat-heads(y) @ Wo^T + bo.

Sharding: 8 cores = 4 batches x 2 head-halves. Core c handles batch c//2 and
heads (c%2)*8 .. (c%2)*8+7. Host sums the two half partials per batch and adds
the folded bias (bv folds through softmax into bo; bk is softmax-invariant).

Precision/throughput scheme:
  - Projections (A) and out-projection (C) run as fp8 DoubleRow matmuls at
    0.5 cycles/row with 2 contraction tiles packed per instruction. Full
    precision is recovered with a 3-term hi/lo decomposition:
        W@X ~= Whi@Xhi + Wlo@Xhi + Whi@Xlo
    where hi = e4m3(a), lo = a - hi (e5m2 for weights, whose lo falls below
    e4m3's subnormal range; e4m3 for activations). Cost is 0.75x of an
    fp32r/bf16 matmul at ~bf16-level accuracy.
  - x and all weights are hi/lo-split on the host. y (out-proj input) is
    split on-device: DVE writes y*1/r as fp32 scratch, ACT casts to e4m3 hi,
    DVE subtracts for the e4m3 lo.
  - Attention (B) runs in bf16: scores k-stationary/q-moving pre-transposed
    per key tile so exp writes the P@V moving operand straight to SBUF; row
    sums ride as an all-ones matmul next to y; 1/r is applied at y eviction.
    v is produced already [t,e]-transposed by Phase A (x-tiles stationary),
    so Phase B needs no PE transposes. exp absorbs the 1/sqrt(E) scale.
"""

import numpy as np

B, T, D, H = 4, 1024, 2048, 16
E = D // H            # 128
P = 128
ND = D // P           # 16 d-tiles
NP = ND // 2          # 8 d-tile pairs (DoubleRow)
NT = T // P           # 8 t-blocks / q-blocks / k-tiles
HL = H // 2           # 8 heads per core
NPC = HL // 2         # 4 i-tile pairs in out-proj
SCALE = 1.0 / np.sqrt(E)

_cache = {}


def _build():
    import concourse.bass as bass
    import concourse.mybir as mybir
    import concourse.tile as tile
    from concourse import bacc
    from concourse.bass import ts

    F32 = mybir.dt.float32
    BF16 = mybir.dt.bfloat16
    F8E4 = mybir.dt.float8e4
    F8E5 = mybir.dt.float8e5
    AF = mybir.ActivationFunctionType
    OP = mybir.AluOpType
    DR = mybir.MatmulPerfMode.DoubleRow

    nc = bacc.Bacc("TRN2", target_bir_lowering=False, debug=False)

    xh_d = nc.dram_tensor("xh", [P, ND, T], F8E4, kind="ExternalInput").ap()
    xl_d = nc.dram_tensor("xl", [P, ND, T], F8E4, kind="ExternalInput").ap()
    w3h_d = nc.dram_tensor("w3h", [HL, P, NP, 2, 3 * E], F8E4,
                           kind="ExternalInput").ap()
    w3l_d = nc.dram_tensor("w3l", [HL, P, NP, 2, 3 * E], F8E5,
                           kind="ExternalInput").ap()
    bqT_d = nc.dram_tensor("bqT", [P, HL], F32, kind="ExternalInput").ap()
    woth_d = nc.dram_tensor("woth", [ND, P, NPC, 2, P], F8E4,
                            kind="ExternalInput").ap()
    wotl_d = nc.dram_tensor("wotl", [ND, P, NPC, 2, P], F8E5,
                            kind="ExternalInput").ap()
    outT_d = nc.dram_tensor("outT", [ND, P, T], F32, kind="ExternalOutput").ap()

    QS, KS, VS = slice(0, E), slice(E, 2 * E), slice(2 * E, 3 * E)

    with tile.TileContext(nc) as tc:
        with (
            tc.tile_pool(name="const", bufs=1) as const,
            tc.tile_pool(name="qkv", bufs=1) as qkv,
            tc.tile_pool(name="small", bufs=4) as small,
        ):
            ones_f = const.tile([P, P], F32)
            nc.vector.memset(ones_f[:], 1.0)
            ones_b = const.tile([P, P], BF16)
            nc.vector.tensor_copy(ones_b[:], ones_f[:])
            bqT_t = const.tile([P, HL], F32)
            # all-zero fp8 operands: a full-bank "zero matmul" opens each psv
            # PSUM bank exactly once (one start per 2KB zero region; psv's
            # per-t-block groups then accumulate without further starts)
            zeros_t = const.tile([P, 2, 512], F8E4)
            nc.vector.memset(zeros_t[:], 0.0)

            qT = qkv.tile([P, HL, T], BF16)          # [e, head, t]
            kT = qkv.tile([P, HL, T], BF16)
            vA = qkv.tile([P, HL, NT, E], BF16)      # [t-in-block, head, tb, e]
            yh = qkv.tile([P, HL, T], F8E4)          # y hi  [e, head, t]
            yl = qkv.tile([P, HL, T], F8E4)          # y lo

            # ---------------- Phase A: q/k/v projections (fp8 DR) ----------
            with (
                tc.tile_pool(name="xp", bufs=1) as xp,
                tc.tile_pool(name="w3p", bufs=2) as w3p,
                tc.tile_pool(name="ps_a", bufs=4, space="PSUM") as ps_a,
            ):
                xh_t = xp.tile([P, ND, T], F8E4)
                xl_t = xp.tile([P, ND, T], F8E4)

                def w3_dma(hl, split=False):
                    w3h_t = w3p.tile([P, NP, 2, 3 * E], F8E4, tag="w3h",
                                     name="w3h_t")
                    w3l_t = w3p.tile([P, NP, 2, 3 * E], F8E5, tag="w3l",
                                     name="w3l_t")
                    if split:  # head 0: per-pair groups so compute
                        # starts early; caller issues the w3l DMA later
                        for g in range(NP):
                            nc.sync.dma_start(w3h_t[:, g:g + 1],
                                              w3h_d[hl][:, g:g + 1])
                    else:
                        nc.sync.dma_start(w3h_t[:], w3h_d[hl])
                        nc.sync.dma_start(w3l_t[:], w3l_d[hl])
                    return w3h_t, w3l_t

                # DMA order: xh pairs + w3h(h0) interleaved (main term streams),
                # then w3l(h0) so wcorr runs full-speed while xl streams last.
                nc.sync.dma_start(xh_t[:, 0:2, :], xh_d[:, 0:2, :])
                w3_h0 = w3_dma(0, split=True)
                for dt in range(2, ND, 2):
                    nc.sync.dma_start(xh_t[:, dt:dt + 2, :], xh_d[:, dt:dt + 2, :])
                nc.sync.dma_start(w3_h0[1][:], w3l_d[0])
                for dt in range(0, ND, 4):
                    nc.sync.dma_start(xl_t[:, dt:dt + 4, :], xl_d[:, dt:dt + 4, :])
                nc.sync.dma_start(bqT_t[:], bqT_d)
                # preload the ACT Exp table off the critical path
                dummy = small.tile([P, 1], F32, tag="racc", name="dummy")
                nc.scalar.activation(dummy[:], bqT_t[:, 0:1], AF.Exp)

                for hl in range(HL):
                    w3h_t, w3l_t = w3_h0 if hl == 0 else w3_nx[0]
                    psq = ps_a.tile([P, T], F32, tag="a")
                    psk = ps_a.tile([P, T], F32, tag="a")
                    psv = ps_a.tile([P, NT, E], F32, tag="a")
                    for bank in range(2):
                        nc.tensor.matmul(
                            psv[:, 4 * bank:4 * bank + 4, :],
                            zeros_t[:, :, 0:P], zeros_t[:],
                            start=True, stop=False, perf_mode=DR,
                            skip_group_check=True)

                    # terms: (stationary-w, moving-x) for q/k;
                    #        (stationary-x, moving-w) for v
                    terms = [(w3h_t, xh_t), (w3l_t, xh_t), (w3h_t, xl_t)]

                    for ti, (wt, xt) in enumerate(terms):
                        st, sp = ti == 0, ti == 2
                        for pr in range(NP):
                            s0, sN = st and pr == 0, sp and pr == NP - 1
                            xpair = xt[:, 2 * pr:2 * pr + 2, :]
                            for c in range(2):
                                nc.tensor.matmul(
                                    psq[:, ts(c, 512)], wt[:, pr, :, QS],
                                    xpair[:, :, ts(c, 512)], start=s0, stop=sN,
                                    perf_mode=DR, skip_group_check=True)
                                nc.tensor.matmul(
                                    psk[:, ts(c, 512)], wt[:, pr, :, KS],
                                    xpair[:, :, ts(c, 512)], start=s0, stop=sN,
                                    perf_mode=DR, skip_group_check=True)
                            for tb in range(NT):
                                nc.tensor.matmul(
                                    psv[:, tb, :], xpair[:, :, ts(tb, P)],
                                    wt[:, pr, :, VS], start=False,
                                    stop=sN and tb % 4 == 3,
                                    perf_mode=DR, skip_group_check=True)

                    # prefetch next head's weights while this head computes
                    if hl + 1 < HL:
                        w3_nx = [w3_dma(hl + 1)]

                    # evict: q gets +bq on DVE; k on ACT; v on DVE (bf16 out)
                    nc.vector.tensor_scalar(
                        qT[:, hl, :], psq[:], bqT_t[:, hl:hl + 1], None,
                        op0=OP.add)
                    nc.scalar.activation(kT[:, hl, :], psk[:], AF.Copy)
                    nc.vector.tensor_copy(vA[:, hl], psv[:])

            # ---------------- Phases B+C scope ----------------
            with (
                tc.tile_pool(name="wop", bufs=3) as wop,
            ):
                def wot_dma(ob):
                    woth_t = wop.tile([P, NPC, 2, P], F8E4, tag="woh",
                                      name="woth_t")
                    wotl_t = wop.tile([P, NPC, 2, P], F8E5, tag="wol",
                                      name="wotl_t")
                    nc.sync.dma_start(woth_t[:], woth_d[ob])
                    nc.sync.dma_start(wotl_t[:], wotl_d[ob])
                    return woth_t, wotl_t

                wot_pre = [wot_dma(ob) for ob in range(2)]

                # -------- Phase B: attention per head (bf16), software-
                # pipelined across heads: the next head's first two score
                # rows (and their exp) are emitted inside the current head's
                # AV tail, and the chunk-1 AV stream lags chunk-0 by one
                # k-tile so the y-normalization DVE chain never gates the
                # next head's first AV matmuls. --------
                with (
                    tc.tile_pool(name="etp", bufs=2) as etp,
                    tc.tile_pool(name="rbp", bufs=2) as rbp,
                    tc.tile_pool(name="yfp", bufs=2) as yfp,
                    tc.tile_pool(name="ps_s", bufs=2, space="PSUM") as ps_s,
                    tc.tile_pool(name="ps_ar", bufs=2, space="PSUM") as ps_ar,
                ):
                    def make_head(hl):
                        return {
                            "hl": hl,
                            "ET": etp.tile([P, NT, T], BF16, tag="ET", name="ET"),
                            "rb": rbp.tile([P, T], F32, tag="rb", name="rb"),
                            "y0": ps_ar.tile([P, 512], F32, tag="y", name="y0"),
                            "r0": ps_ar.tile([P, 512], F32, tag="r", name="r0"),
                            "y1": ps_ar.tile([P, 512], F32, tag="y", name="y1"),
                            "r1": ps_ar.tile([P, 512], F32, tag="r", name="r1"),
                        }

                    def emit_ST(h, j):
                        # scores for k-tile j in one [P, T] psum so a single
                        # exp covers the whole causal row range
                        hl, ET = h["hl"], h["ET"]
                        kblk = kT[:, hl, ts(j, P)]
                        s_ps = ps_s.tile([P, T], F32, tag="s", name="s_ps")
                        if j < 4:
                            nc.tensor.matmul(s_ps[:, j * P:512], kblk,
                                             qT[:, hl, j * P:512],
                                             start=True, stop=True)
                            nc.tensor.matmul(s_ps[:, 512:T], kblk,
                                             qT[:, hl, 512:T],
                                             start=True, stop=True)
                        else:
                            nc.tensor.matmul(s_ps[:, j * P:T], kblk,
                                             qT[:, hl, j * P:T],
                                             start=True, stop=True)
                        nc.scalar.activation(ET[:, j, j * P:T],
                                             s_ps[:, j * P:T], AF.Exp,
                                             scale=float(SCALE))
                        nc.gpsimd.affine_select(
                            out=ET[:, j, j * P:(j + 1) * P],
                            in_=ET[:, j, j * P:(j + 1) * P],
                            compare_op=OP.is_ge, fill=0.0,
                            base=0, pattern=[[1, P]], channel_multiplier=-1,
                        )

                    def emit_AV0(h, jq):  # q-chunk 0: cols jq*P..512, jq<=3
                        lo = jq * P
                        st, sp = jq == 0, jq == 3
                        ET = h["ET"]
                        nc.tensor.matmul(h["y0"][:, lo:512],
                                         vA[:, h["hl"], jq, :],
                                         ET[:, jq, lo:512], start=st, stop=sp,
                                         skip_group_check=True)
                        nc.tensor.matmul(h["r0"][:, lo:512], ones_b[:],
                                         ET[:, jq, lo:512], start=st, stop=sp,
                                         skip_group_check=True)

                    def emit_AV1(h, jq):  # q-chunk 1: cols 512..T, all jq
                        lo = max(jq * P, 512)
                        st, sp = jq == 0, jq == NT - 1
                        ET = h["ET"]
                        nc.tensor.matmul(h["y1"][:, lo - 512:512],
                                         vA[:, h["hl"], jq, :],
                                         ET[:, jq, lo:T], start=st, stop=sp,
                                         skip_group_check=True)
                        nc.tensor.matmul(h["r1"][:, lo - 512:512], ones_b[:],
                                         ET[:, jq, lo:T], start=st, stop=sp,
                                         skip_group_check=True)

                    def emit_ynorm(h, c, y, r):
                        # yf = y * (1/r) f32; hi = e4m3(yf); lo = yf - hi
                        # (all on DVE: ACT is the busier engine in B)
                        hl, rb = h["hl"], h["rb"]
                        yf = yfp.tile([P, 512], F32, tag="yf", name="yf")
                        nc.vector.reciprocal(rb[:, ts(c, 512)], r[:])
                        nc.vector.tensor_mul(yf[:], y[:], rb[:, ts(c, 512)])
                        nc.vector.tensor_copy(yh[:, hl, ts(c, 512)], yf[:])
                        nc.vector.tensor_tensor(
                            yl[:, hl, ts(c, 512)], yf[:],
                            yh[:, hl, ts(c, 512)], op=OP.subtract)

                    cur = make_head(0)
                    emit_ST(cur, 0)
                    emit_ST(cur, 1)
                    for hl in range(HL):
                        for j in range(2, NT):
                            emit_ST(cur, j)
                            if j - 2 <= 3:
                                emit_AV0(cur, j - 2)
                            if j >= 3:
                                emit_AV1(cur, j - 3)
                            if j == 5:
                                emit_ynorm(cur, 0, cur["y0"], cur["r0"])
                        emit_AV1(cur, 5)
                        if hl + 1 < HL:
                            nxt = make_head(hl + 1)
                            emit_ST(nxt, 0)
                            emit_ST(nxt, 1)
                        emit_AV1(cur, 6)
                        emit_AV1(cur, 7)
                        emit_ynorm(cur, 1, cur["y1"], cur["r1"])
                        if hl + 1 < HL:
                            cur = nxt

                # -------- Phase C: partial out-projection (fp8 DR) --------
                with (
                    tc.tile_pool(name="osb", bufs=2) as osb,
                    tc.tile_pool(name="ps_o", bufs=2, space="PSUM") as ps_o,
                ):
                    for ob in range(ND):
                        woth_t, wotl_t = wot_pre[ob] if ob < 2 else wot_dma(ob)
                        o_ps = ps_o.tile([P, T], F32, tag="o")
                        out_sb = osb.tile([P, T], F32, tag="osb")
                        for c in range(2):
                            n = 0
                            for wt, yt in ((woth_t, yh), (wotl_t, yh),
                                           (woth_t, yl)):
                                for pr in range(NPC):
                                    nc.tensor.matmul(
                                        o_ps[:, ts(c, 512)], wt[:, pr],
                                        yt[:, 2 * pr:2 * pr + 2, ts(c, 512)],
                                        start=n == 0, stop=n == 3 * NPC - 1,
                                        perf_mode=DR, skip_group_check=True)
                                    n += 1
                            # per-chunk evict + DMA so the store pipeline
                            # overlaps the second chunk's matmuls
                            nc.scalar.activation(out_sb[:, ts(c, 512)],
                                                 o_ps[:, ts(c, 512)], AF.Copy)
                            nc.sync.dma_start(outT_d[ob][:, ts(c, 512)],
                                              out_sb[:, ts(c, 512)])

    nc.compile()
    return nc


def _get_compiled():
    if "nc" not in _cache:
        _cache["nc"] = _build()
    return _cache["nc"]


def _hilo(a, lo_dt):
    import ml_dtypes
    hi = np.ascontiguousarray(a).astype(ml_dtypes.float8_e4m3)
    lo = (a - hi.astype(np.float32)).astype(lo_dt)
    return hi, lo


def _host_prep(x, Wq, bq, Wk, Wv, Wo):
    """Build per-core input maps (hi/lo fp8 splits + DR pair packing)."""
    import ml_dtypes
    E4, E5 = ml_dtypes.float8_e4m3, ml_dtypes.float8_e5m2
    in_maps = []
    xs = []
    for b in range(B):
        # [P, ND, T]: row p holds d-tile-major slices, matching the SBUF tile
        xT = np.ascontiguousarray(
            x[b].T.reshape(ND, P, T).transpose(1, 0, 2))
        xs.append(_hilo(xT, E4))
    halves = []
    for half in range(2):
        hs = slice(half * HL, (half + 1) * HL)
        # w3 packed [HL, P, NP, 2, 3E]: slot s of pair pr = d-tile 2pr+s
        w3 = np.empty((HL, P, NP, 2, 3 * E), dtype=np.float32)
        for hl, h in enumerate(range(half * HL, (half + 1) * HL)):
            for j, W in enumerate((Wq[h], Wk[h], Wv[h])):
                wt = W.T.reshape(NP, 2, P, E)          # [pr, s, p(d), e]
                w3[hl, :, :, :, j * E:(j + 1) * E] = wt.transpose(2, 0, 1, 3)
        w3h, w3l = _hilo(w3, E5)
        bqT = np.ascontiguousarray(bq[hs].T)           # [E, HL]
        # wot [ND, P, NPC, 2, P]: [ob, i-in-tile, pr, s, o], i-tile = 2pr+s
        WoT_span = Wo.T[half * 1024:(half + 1) * 1024]  # [1024, D]
        wot = np.ascontiguousarray(
            WoT_span.reshape(NPC, 2, P, ND, P).transpose(3, 2, 0, 1, 4))
        woth, wotl = _hilo(wot, E5)
        halves.append({"w3h": w3h, "w3l": w3l, "bqT": bqT,
                       "woth": woth, "wotl": wotl})
    for c in range(8):
        b, half = c // 2, c % 2
        hv = halves[half]
        in_maps.append({"xh": xs[b][0], "xl": xs[b][1], **hv})
    return in_maps


def _numpy_fallback(x, attention_mask, Wq, bq, Wk, bk, Wv, bv, Wo, bo):
    out = np.empty((B, T, D), dtype=np.float32)
    neg = np.float32(np.finfo(np.float32).min)
    for b in range(B):
        xb = x[b]
        q = np.einsum("td,hed->hte", xb, Wq) + bq[:, None, :]
        k = np.einsum("td,hed->hte", xb, Wk) + bk[:, None, :]
        v = np.einsum("td,hed->hte", xb, Wv) + bv[:, None, :]
        s = np.einsum("hqe,hke->hqk", q, k).astype(np.float32) * np.float32(SCALE)
        causal = np.arange(T)[None, :] > np.arange(T)[:, None]
        s = np.where(causal[None], neg, s)
        keep = attention_mask[b].astype(bool)
        s = np.where(keep[None, None, :], s, neg)
        s = s - s.max(-1, keepdims=True)
        p = np.exp(s)
        p = p / p.sum(-1, keepdims=True)
        y = np.einsum("hqk,hke->hqe", p, v)
        y = np.transpose(y, (1, 0, 2)).reshape(T, D)
        out[b] = y @ Wo.T + bo
    return out


def kernel(x, attention_mask, Wq, bq, Wk, bk, Wv, bv, Wo, bo):
    x = np.asarray(x, dtype=np.float32)
    attention_mask = np.asarray(attention_mask)
    Wq, bq = np.asarray(Wq, np.float32), np.asarray(bq, np.float32)
    Wk, bk = np.asarray(Wk, np.float32), np.asarray(bk, np.float32)
    Wv, bv = np.asarray(Wv, np.float32), np.asarray(bv, np.float32)
    Wo, bo = np.asarray(Wo, np.float32), np.asarray(bo, np.float32)

    if not np.all(attention_mask == 1):
        return _numpy_fallback(x, attention_mask, Wq, bq, Wk, bk, Wv, bv, Wo, bo)

    from concourse.bass_utils import run_bass_kernel_spmd

    nc = _get_compiled()
    in_maps = _host_prep(x, Wq, bq, Wk, Wv, Wo)
    res = run_bass_kernel_spmd(nc, in_maps, core_ids=list(range(8)))

    # bv folds through softmax (rows sum to 1); bk is softmax-invariant
    bo_total = (bo + Wo @ bv.reshape(D)).astype(np.float32)

    out = np.zeros((B, T, D), dtype=np.float32)
    for c in range(8):
        partial = res.results[c]["outT"].reshape(D, T)  # [o, t]
        out[c // 2] += partial.T
    out += bo_total
    return out


# revision 20
# speedup vs baseline: 1.2847x; 1.0191x over previous
"""Trainium2 Bass kernel for multi-head causal self-attention.

Problem: B=4, T=1024, D=2048, H=16 heads, E=128 head_dim, fp32 I/O.
  q/k/v = per-head projections of x; scores = causal-masked softmax(q k^T / sqrt(E));
  y = probs @ v; out = concat-heads(y) @ Wo^T + bo.

Sharding: 8 cores = 4 batches x 2 head-halves. Core c handles batch c//2 and
heads (c%2)*8 .. (c%2)*8+7. Host sums the two half partials per batch and adds
the folded bias (bv folds through softmax into bo; bk is softmax-invariant).

Precision/throughput scheme:
  - Projections (A) and out-projection (C) run as fp8 DoubleRow matmuls at
    0.5 cycles/row with 2 contraction tiles packed per instruction. Full
    precision is recovered with a 3-term hi/lo decomposition:
        W@X ~= Whi@Xhi + Wlo@Xhi + Whi@Xlo
    where hi = e4m3(a), lo = a - hi (e5m2 for weights, whose lo falls below
    e4m3's subnormal range; e4m3 for activations). Cost is 0.75x of an
    fp32r/bf16 matmul at ~bf16-level accuracy.
  - x and all weights are hi/lo-split on the host. y (out-proj input) is
    split on-device: DVE writes y*1/r as fp32 scratch, ACT casts to e4m3 hi,
    DVE subtracts for the e4m3 lo.
  - Attention (B) runs in bf16: scores k-stationary/q-moving pre-transposed
    per key tile so exp writes the P@V moving operand straight to SBUF; row
    sums ride as an all-ones matmul next to y; 1/r is applied at y eviction.
    v is produced already [t,e]-transposed by Phase A (x-tiles stationary),
    so Phase B needs no PE transposes. exp absorbs the 1/sqrt(E) scale.
"""

import numpy as np

B, T, D, H = 4, 1024, 2048, 16
E = D // H            # 128
P = 128
ND = D // P           # 16 d-tiles
NP = ND // 2          # 8 d-tile pairs (DoubleRow)
NT = T // P           # 8 t-blocks / q-blocks / k-tiles
HL = H // 2           # 8 heads per core
NPC = HL // 2         # 4 i-tile pairs in out-proj
SCALE = 1.0 / np.sqrt(E)

_cache = {}


def _build():
    import concourse.bass as bass
    import concourse.mybir as mybir
    import concourse.tile as tile
    from concourse import bacc
    from concourse.bass import ts

    F32 = mybir.dt.float32
    BF16 = mybir.dt.bfloat16
    F8E4 = mybir.dt.float8e4
    F8E5 = mybir.dt.float8e5
    AF = mybir.ActivationFunctionType
    OP = mybir.AluOpType
    DR = mybir.MatmulPerfMode.DoubleRow

    nc = bacc.Bacc("TRN2", target_bir_lowering=False, debug=False)

    xh_d = nc.dram_tensor("xh", [P, ND, T], F8E4, kind="ExternalInput").ap()
    xl_d = nc.dram_tensor("xl", [P, ND, T], F8E4, kind="ExternalInput").ap()
    w3h_d = nc.dram_tensor("w3h", [HL, P, NP, 2, 3 * E], F8E4,
                           kind="ExternalInput").ap()
    w3l_d = nc.dram_tensor("w3l", [HL, P, NP, 2, 3 * E], F8E5,
                           kind="ExternalInput").ap()
    bqT_d = nc.dram_tensor("bqT", [P, HL], F32, kind="ExternalInput").ap()
    woth_d = nc.dram_tensor("woth", [ND, P, NPC, 2, P], F8E4,
                            kind="ExternalInput").ap()
    wotl_d = nc.dram_tensor("wotl", [ND, P, NPC, 2, P], F8E5,
                            kind="ExternalInput").ap()
    F16 = mybir.dt.float16
    outT_d = nc.dram_tensor("outT", [ND, P, T], F16,
                            kind="ExternalOutput").ap()

    QS, KS, VS = slice(0, E), slice(E, 2 * E), slice(2 * E, 3 * E)

    with tile.TileContext(nc) as tc:
        with (
            tc.tile_pool(name="const", bufs=1) as const,
            tc.tile_pool(name="qkv", bufs=1) as qkv,
            tc.tile_pool(name="small", bufs=4) as small,
        ):
            ones_f = const.tile([P, P], F32)
            nc.vector.memset(ones_f[:], 1.0)
            ones_b = const.tile([P, P], BF16)
            nc.vector.tensor_copy(ones_b[:], ones_f[:])
            bqT_t = const.tile([P, HL], F32)
            # all-zero fp8 operands: a full-bank "zero matmul" opens each psv
            # PSUM bank exactly once (one start per 2KB zero region; psv's
            # per-t-block groups then accumulate without further starts)
            zeros_t = const.tile([P, 2, 512], F8E4)
            nc.vector.memset(zeros_t[:], 0.0)

            qT = qkv.tile([P, HL, T], BF16)          # [e, head, t]
            kT = qkv.tile([P, HL, T], BF16)
            vA = qkv.tile([P, HL, NT, E], BF16)      # [t-in-block, head, tb, e]
            yh = qkv.tile([P, HL, T], F8E4)          # y hi  [e, head, t]
            yl = qkv.tile([P, HL, T], F8E4)          # y lo

            # ---------------- Phase A: q/k/v projections (fp8 DR) ----------
            with (
                tc.tile_pool(name="xp", bufs=1) as xp,
                tc.tile_pool(name="w3p", bufs=2) as w3p,
                tc.tile_pool(name="ps_a", bufs=4, space="PSUM") as ps_a,
            ):
                xh_t = xp.tile([P, ND, T], F8E4)
                xl_t = xp.tile([P, ND, T], F8E4)

                def w3_dma(hl, split=False):
                    w3h_t = w3p.tile([P, NP, 2, 3 * E], F8E4, tag="w3h",
                                     name="w3h_t")
                    w3l_t = w3p.tile([P, NP, 2, 3 * E], F8E5, tag="w3l",
                                     name="w3l_t")
                    if not split:  # split: caller issues per-pair DMAs
                        nc.sync.dma_start(w3h_t[:], w3h_d[hl])
                        nc.sync.dma_start(w3l_t[:], w3l_d[hl])
                    return w3h_t, w3l_t

                # DMA order follows head-0 consumption: per-pair xh+w3h
                # (main term), then w3l (wcorr), then xl (xcorr).
                w3_h0 = w3_dma(0, split=True)
                for pr in range(NP):
                    nc.sync.dma_start(xh_t[:, 2 * pr:2 * pr + 2, :],
                                      xh_d[:, 2 * pr:2 * pr + 2, :])
                    nc.sync.dma_start(w3_h0[0][:, pr:pr + 1],
                                      w3h_d[0][:, pr:pr + 1])
                nc.sync.dma_start(w3_h0[1][:], w3l_d[0])
                for dt in range(0, ND, 4):
                    nc.sync.dma_start(xl_t[:, dt:dt + 4, :], xl_d[:, dt:dt + 4, :])
                nc.sync.dma_start(bqT_t[:], bqT_d)
                # preload the ACT Exp table off the critical path
                dummy = small.tile([P, 1], F32, tag="racc", name="dummy")
                nc.scalar.activation(dummy[:], bqT_t[:, 0:1], AF.Exp)

                for hl in range(HL):
                    w3h_t, w3l_t = w3_h0 if hl == 0 else w3_nx[0]
                    psq = ps_a.tile([P, T], F32, tag="a")
                    psk = ps_a.tile([P, T], F32, tag="a")
                    psv = ps_a.tile([P, NT, E], F32, tag="a")
                    for bank in range(2):
                        nc.tensor.matmul(
                            psv[:, 4 * bank:4 * bank + 4, :],
                            zeros_t[:, :, 0:P], zeros_t[:],
                            start=True, stop=False, perf_mode=DR,
                            skip_group_check=True)

                    # terms: (stationary-w, moving-x) for q/k;
                    #        (stationary-x, moving-w) for v
                    terms = [(w3h_t, xh_t), (w3l_t, xh_t), (w3h_t, xl_t)]

                    for ti, (wt, xt) in enumerate(terms):
                        st, sp = ti == 0, ti == 2
                        for pr in range(NP):
                            s0, sN = st and pr == 0, sp and pr == NP - 1
                            xpair = xt[:, 2 * pr:2 * pr + 2, :]
                            for c in range(2):
                                nc.tensor.matmul(
                                    psq[:, ts(c, 512)], wt[:, pr, :, QS],
                                    xpair[:, :, ts(c, 512)], start=s0, stop=sN,
                                    perf_mode=DR, skip_group_check=True)
                                nc.tensor.matmul(
                                    psk[:, ts(c, 512)], wt[:, pr, :, KS],
                                    xpair[:, :, ts(c, 512)], start=s0, stop=sN,
                                    perf_mode=DR, skip_group_check=True)
                            for tb in range(NT):
                                nc.tensor.matmul(
                                    psv[:, tb, :], xpair[:, :, ts(tb, P)],
                                    wt[:, pr, :, VS], start=False,
                                    stop=sN and tb % 4 == 3,
                                    perf_mode=DR, skip_group_check=True)

                    # prefetch next head's weights while this head computes
                    if hl + 1 < HL:
                        w3_nx = [w3_dma(hl + 1)]

                    # evict: q gets +bq on DVE; k on ACT; v on DVE (bf16 out)
                    nc.vector.tensor_scalar(
                        qT[:, hl, :], psq[:], bqT_t[:, hl:hl + 1], None,
                        op0=OP.add)
                    nc.scalar.activation(kT[:, hl, :], psk[:], AF.Copy)
                    nc.vector.tensor_copy(vA[:, hl, 0:NT // 2], psv[:, 0:NT // 2])
                    nc.scalar.activation(vA[:, hl, NT // 2:NT],
                                         psv[:, NT // 2:NT], AF.Copy)

            # ---------------- Phases B+C scope ----------------
            with (
                tc.tile_pool(name="wop", bufs=3) as wop,
            ):
                def wot_dma(ob):
                    woth_t = wop.tile([P, NPC, 2, P], F8E4, tag="woh",
                                      name="woth_t")
                    wotl_t = wop.tile([P, NPC, 2, P], F8E5, tag="wol",
                                      name="wotl_t")
                    nc.sync.dma_start(woth_t[:], woth_d[ob])
                    nc.sync.dma_start(wotl_t[:], wotl_d[ob])
                    return woth_t, wotl_t

                wot_pre = [wot_dma(ob) for ob in range(2)]

                # -------- Phase B: attention per head (bf16), software-
                # pipelined across heads: the next head's first two score
                # rows (and their exp) are emitted inside the current head's
                # AV tail, and the chunk-1 AV stream lags chunk-0 by one
                # k-tile so the y-normalization DVE chain never gates the
                # next head's first AV matmuls. --------
                with (
                    tc.tile_pool(name="etp", bufs=2) as etp,
                    tc.tile_pool(name="rbp", bufs=2) as rbp,
                    tc.tile_pool(name="yfp", bufs=2) as yfp,
                    tc.tile_pool(name="ps_s", bufs=2, space="PSUM") as ps_s,
                    tc.tile_pool(name="ps_ar", bufs=2, space="PSUM") as ps_ar,
                ):
                    def make_head(hl):
                        return {
                            "hl": hl,
                            "ET": etp.tile([P, NT, T], BF16, tag="ET", name="ET"),
                            "rb": rbp.tile([P, T], F32, tag="rb", name="rb"),
                            "y0": ps_ar.tile([P, 512], F32, tag="y", name="y0"),
                            "r0": ps_ar.tile([P, 512], F32, tag="r", name="r0"),
                            "y1": ps_ar.tile([P, 512], F32, tag="y", name="y1"),
                            "r1": ps_ar.tile([P, 512], F32, tag="r", name="r1"),
                        }

                    def emit_ST(h, j):
                        # scores for k-tile j in one [P, T] psum so a single
                        # exp covers the whole causal row range
                        hl, ET = h["hl"], h["ET"]
                        kblk = kT[:, hl, ts(j, P)]
                        s_ps = ps_s.tile([P, T], F32, tag="s", name="s_ps")
                        if j < 4:
                            nc.tensor.matmul(s_ps[:, j * P:512], kblk,
                                             qT[:, hl, j * P:512],
                                             start=True, stop=True)
                            nc.tensor.matmul(s_ps[:, 512:T], kblk,
                                             qT[:, hl, 512:T],
                                             start=True, stop=True)
                        else:
                            nc.tensor.matmul(s_ps[:, j * P:T], kblk,
                                             qT[:, hl, j * P:T],
                                             start=True, stop=True)
                        nc.scalar.activation(ET[:, j, j * P:T],
                                             s_ps[:, j * P:T], AF.Exp,
                                             scale=float(SCALE))
                        nc.gpsimd.affine_select(
                            out=ET[:, j, j * P:(j + 1) * P],
                            in_=ET[:, j, j * P:(j + 1) * P],
                            compare_op=OP.is_ge, fill=0.0,
                            base=0, pattern=[[1, P]], channel_multiplier=-1,
                        )

                    def emit_AV0(h, jq):  # q-chunk 0: cols jq*P..512, jq<=3
                        lo = jq * P
                        st, sp = jq == 0, jq == 3
                        ET = h["ET"]
                        nc.tensor.matmul(h["y0"][:, lo:512],
                                         vA[:, h["hl"], jq, :],
                                         ET[:, jq, lo:512], start=st, stop=sp,
                                         skip_group_check=True)
                        nc.tensor.matmul(h["r0"][:, lo:512], ones_b[:],
                                         ET[:, jq, lo:512], start=st, stop=sp,
                                         skip_group_check=True)

                    def emit_AV1(h, jq):  # q-chunk 1: cols 512..T, all jq
                        lo = max(jq * P, 512)
                        st, sp = jq == 0, jq == NT - 1
                        ET = h["ET"]
                        nc.tensor.matmul(h["y1"][:, lo - 512:512],
                                         vA[:, h["hl"], jq, :],
                                         ET[:, jq, lo:T], start=st, stop=sp,
                                         skip_group_check=True)
                        nc.tensor.matmul(h["r1"][:, lo - 512:512], ones_b[:],
                                         ET[:, jq, lo:T], start=st, stop=sp,
                                         skip_group_check=True)

                    def emit_ynorm(h, c, y, r):
                        # yf = y * (1/r) f32; hi = e4m3(yf); lo = yf - hi
                        # (all on DVE: ACT is the busier engine in B)
                        hl, rb = h["hl"], h["rb"]
                        yf = yfp.tile([P, 512], F32, tag="yf", name="yf")
                        nc.vector.reciprocal(rb[:, ts(c, 512)], r[:])
                        nc.vector.tensor_mul(yf[:], y[:], rb[:, ts(c, 512)])
                        nc.vector.tensor_copy(yh[:, hl, ts(c, 512)], yf[:])
                        nc.vector.tensor_tensor(
                            yl[:, hl, ts(c, 512)], yf[:],
                            yh[:, hl, ts(c, 512)], op=OP.subtract)

                    cur = make_head(0)
                    emit_ST(cur, 0)
                    emit_ST(cur, 1)
                    emit_ST(cur, 2)
                    for hl in range(HL):
                        for j in range(2, NT):
                            if not (hl == 0 and j == 2):
                                emit_ST(cur, j)
                            if j - 2 <= 3:
                                emit_AV0(cur, j - 2)
                            if j >= 5:
                                emit_AV1(cur, j - 5)
                            if j == 5:
                                emit_ynorm(cur, 0, cur["y0"], cur["r0"])
                        emit_AV1(cur, 3)
                        emit_AV1(cur, 4)
                        if hl + 1 < HL:
                            nxt = make_head(hl + 1)
                            emit_ST(nxt, 0)
                            emit_ST(nxt, 1)
                        emit_AV1(cur, 5)
                        emit_AV1(cur, 6)
                        emit_AV1(cur, 7)
                        emit_ynorm(cur, 1, cur["y1"], cur["r1"])
                        if hl + 1 < HL:
                            cur = nxt

                # -------- Phase C: partial out-projection (fp8 DR) --------
                with (
                    tc.tile_pool(name="osb", bufs=2) as osb,
                    tc.tile_pool(name="ps_o", bufs=2, space="PSUM") as ps_o,
                ):
                    for ob in range(ND):
                        woth_t, wotl_t = wot_pre[ob] if ob < 2 else wot_dma(ob)
                        o_ps = ps_o.tile([P, T], F32, tag="o")
                        out_sb = osb.tile([P, T], F16, tag="osb")
                        for c in range(2):
                            n = 0
                            for wt, yt in ((woth_t, yh), (wotl_t, yh),
                                           (woth_t, yl)):
                                for pr in range(NPC):
                                    nc.tensor.matmul(
                                        o_ps[:, ts(c, 512)], wt[:, pr],
                                        yt[:, 2 * pr:2 * pr + 2, ts(c, 512)],
                                        start=n == 0, stop=n == 3 * NPC - 1,
                                        perf_mode=DR, skip_group_check=True)
                                    n += 1
                            # per-chunk evict + DMA so the store pipeline
                            # overlaps the second chunk's matmuls
                            nc.scalar.activation(out_sb[:, ts(c, 512)],
                                                 o_ps[:, ts(c, 512)], AF.Copy)
                            nc.sync.dma_start(outT_d[ob][:, ts(c, 512)],
                                              out_sb[:, ts(c, 512)])

    nc.compile()
    return nc


def _get_compiled():
    if "nc" not in _cache:
        _cache["nc"] = _build()
    return _cache["nc"]


def _hilo(a, lo_dt):
    import ml_dtypes
    hi = np.ascontiguousarray(a).astype(ml_dtypes.float8_e4m3)
    lo = (a - hi.astype(np.float32)).astype(lo_dt)
    return hi, lo


def _host_prep(x, Wq, bq, Wk, Wv, Wo):
    """Build per-core input maps (hi/lo fp8 splits + DR pair packing)."""
    import ml_dtypes
    E4, E5 = ml_dtypes.float8_e4m3, ml_dtypes.float8_e5m2
    in_maps = []
    xs = []
    for b in range(B):
        # [P, ND, T]: row p holds d-tile-major slices, matching the SBUF tile
        xT = np.ascontiguousarray(
            x[b].T.reshape(ND, P, T).transpose(1, 0, 2))
        xs.append(_hilo(xT, E4))
    halves = []
    for half in range(2):
        hs = slice(half * HL, (half + 1) * HL)
        # w3 packed [HL, P, NP, 2, 3E]: slot s of pair pr = d-tile 2pr+s
        w3 = np.empty((HL, P, NP, 2, 3 * E), dtype=np.float32)
        for hl, h in enumerate(range(half * HL, (half + 1) * HL)):
            for j, W in enumerate((Wq[h], Wk[h], Wv[h])):
                wt = W.T.reshape(NP, 2, P, E)          # [pr, s, p(d), e]
                w3[hl, :, :, :, j * E:(j + 1) * E] = wt.transpose(2, 0, 1, 3)
        w3h, w3l = _hilo(w3, E5)
        bqT = np.ascontiguousarray(bq[hs].T)           # [E, HL]
        # wot [ND, P, NPC, 2, P]: [ob, i-in-tile, pr, s, o], i-tile = 2pr+s
        WoT_span = Wo.T[half * 1024:(half + 1) * 1024]  # [1024, D]
        wot = np.ascontiguousarray(
            WoT_span.reshape(NPC, 2, P, ND, P).transpose(3, 2, 0, 1, 4))
        woth, wotl = _hilo(wot, E5)
        halves.append({"w3h": w3h, "w3l": w3l, "bqT": bqT,
                       "woth": woth, "wotl": wotl})
    for c in range(8):
        b, half = c // 2, c % 2
        hv = halves[half]
        in_maps.append({"xh": xs[b][0], "xl": xs[b][1], **hv})
    return in_maps


def _numpy_fallback(x, attention_mask, Wq, bq, Wk, bk, Wv, bv, Wo, bo):
    out = np.empty((B, T, D), dtype=np.float32)
    neg = np.float32(np.finfo(np.float32).min)
    for b in range(B):
        xb = x[b]
        q = np.einsum("td,hed->hte", xb, Wq) + bq[:, None, :]
        k = np.einsum("td,hed->hte", xb, Wk) + bk[:, None, :]
        v = np.einsum("td,hed->hte", xb, Wv) + bv[:, None, :]
        s = np.einsum("hqe,hke->hqk", q, k).astype(np.float32) * np.float32(SCALE)
        causal = np.arange(T)[None, :] > np.arange(T)[:, None]
        s = np.where(causal[None], neg, s)
        keep = attention_mask[b].astype(bool)
        s = np.where(keep[None, None, :], s, neg)
        s = s - s.max(-1, keepdims=True)
        p = np.exp(s)
        p = p / p.sum(-1, keepdims=True)
        y = np.einsum("hqk,hke->hqe", p, v)
        y = np.transpose(y, (1, 0, 2)).reshape(T, D)
        out[b] = y @ Wo.T + bo
    return out


def kernel(x, attention_mask, Wq, bq, Wk, bk, Wv, bv, Wo, bo):
    x = np.asarray(x, dtype=np.float32)
    attention_mask = np.asarray(attention_mask)
    Wq, bq = np.asarray(Wq, np.float32), np.asarray(bq, np.float32)
    Wk, bk = np.asarray(Wk, np.float32), np.asarray(bk, np.float32)
    Wv, bv = np.asarray(Wv, np.float32), np.asarray(bv, np.float32)
    Wo, bo = np.asarray(Wo, np.float32), np.asarray(bo, np.float32)

    if not np.all(attention_mask == 1):
        return _numpy_fallback(x, attention_mask, Wq, bq, Wk, bk, Wv, bv, Wo, bo)

    from concourse.bass_utils import run_bass_kernel_spmd

    nc = _get_compiled()
    in_maps = _host_prep(x, Wq, bq, Wk, Wv, Wo)
    res = run_bass_kernel_spmd(nc, in_maps, core_ids=list(range(8)))

    # bv folds through softmax (rows sum to 1); bk is softmax-invariant
    bo_total = (bo + Wo @ bv.reshape(D)).astype(np.float32)

    out = np.zeros((B, T, D), dtype=np.float32)
    for c in range(8):
        # fp16 partials off-device; summed here in fp32
        partial = res.results[c]["outT"].astype(np.float32).reshape(D, T)
        out[c // 2] += partial.T
    out += bo_total
    return out
